# revision 33
# baseline (speedup 1.0000x reference)
"""AugmentedTripletLoss Trainium2 kernel — 8-core SPMD, row-sharded.

Math (matches reference):
  d2[i,j] = sq_i + sq_j - 2*X@X.T
  ap_i    = sqrt(clip(max_{same class} d2, 1e-12))
  an_i    = min( sqrt(clip(min_{diff class} d2, 1e-12)),
                 clip(sqrt(clip(sq_i + csq_c - 2*x_i.cn_c, 0)), 1e-12) )
  loss    = mean(relu(1 + ap - an))

Strategy (per core, 512 query rows):
  Host marshals layouts only (transposes / sign scales / one-hot encodes —
  no FLOPs): keys as -X^T fp8 tiles (quarter-blocked for 12KB DMA rows),
  queries as 2*X_q^T fp8, one-hot class aug tiles in bf16.
  Main GEMM runs fp8 DoubleRow (2 contraction subtiles per matmul).
  Row norms are computed on-device from the same fp8 tiles: Scalar
  squares them into bf16, a ones-vector matmul row-reduces into a
  [1,512] PSUM row, and Scalar copies that into the bf16 aug rows
  (sq_j at aug row 96 against query coeff 1; sq_i at aug row 102
  against key coeff 1; BIG*onehot rows complete the aug tile).
  Each [128,1024] PSUM tile then holds u = d2 + BIG*[same class], so
  the masked max/min are plain DVE tensor_reduce passes. Work is
  quarter-pipelined over key columns. Centers: on-device normalize,
  negate, fp8 PE-transpose; csq/sq_i ride the aug rows.
  Final: per-core partial sum -> host gathers the 8 scalars, sums, /N.
"""
import os
import sys

for _p in ("/opt/trn_rl_repo", "/root/.axon_site"):
    if _p not in sys.path:
        sys.path.insert(0, _p)

import numpy as np
import ml_dtypes

import concourse.bass as bass
import concourse.bacc as bacc
import concourse.mybir as mybir
import concourse.bass_isa as bass_isa
from concourse.tile import TileContext
from concourse.masks import make_identity
from concourse.bass_utils import run_bass_kernel_spmd

F32 = mybir.dt.float32
BF16 = mybir.dt.bfloat16
F8 = mybir.dt.float8e4
ALU = mybir.AluOpType
ACTF = mybir.ActivationFunctionType
AX = mybir.AxisListType
DR = mybir.MatmulPerfMode.DoubleRow
NPBF16 = ml_dtypes.bfloat16
NPF8 = ml_dtypes.float8_e4m3

N_CORES = 8
N, D, P = 4096, 768, 100
NQ = N // N_CORES        # 512 query rows per core
MQ = NQ // 128           # 4 query m-tiles
KD = D // 128            # 6 contraction tiles
NQR = 4                  # key-column quarters
QW = N // NQR            # 1024 cols per quarter
BIG = 16384.0
MARGIN = 1.0

_nc_cache = None


def _build():
    nc = bacc.Bacc("TRN2", target_bir_lowering=False, num_devices=N_CORES)

    # xT: quarter-blocked -X^T fp8; col q*(KD*QW) + s*QW + j = -x[q*QW+j, 128s+p]
    xT_h = nc.declare_dram_parameter("xT", [128, KD * N], F8, isOutput=False)
    xq2T_h = nc.declare_dram_parameter("xq2T", [128, KD * NQ], F8, isOutput=False)
    augk_h = nc.declare_dram_parameter("augk", [128, N], BF16, isOutput=False)
    augq_h = nc.declare_dram_parameter("augq", [128, NQ], BF16, isOutput=False)
    augc_h = nc.declare_dram_parameter("augc", [128, 128], BF16, isOutput=False)
    cen_h = nc.declare_dram_parameter("center", [P, D], F32, isOutput=False)
    loss_h = nc.declare_dram_parameter("loss", [1, 1], F32, isOutput=True)

    with TileContext(nc) as tc:
        from contextlib import ExitStack

        with ExitStack() as ctx:
            const = ctx.enter_context(tc.tile_pool(name="const", bufs=1))
            ksqp = ctx.enter_context(tc.tile_pool(name="ksqp", bufs=2))
            pmain = ctx.enter_context(tc.tile_pool(name="pmain", bufs=3, space="PSUM"))
            psmall = ctx.enter_context(tc.tile_pool(name="psmall", bufs=1,
                                                    space="PSUM"))
            pcp = ctx.enter_context(tc.tile_pool(name="pcp", bufs=1,
                                                 space="PSUM"))

            # ---------- persistent tiles ----------
            kT = [const.tile([128, KD, QW], F8, name=f"kT{q}")
                  for q in range(NQR)]                 # -X^T keys, per quarter
            kT6 = const.tile([128, N], BF16)           # aug keys
            qT = const.tile([128, KD, NQ], F8)         # 2*X_q^T query tiles
            qT6 = const.tile([128, NQ], BF16)          # aug queries
            qsq = const.tile([128, KD, NQ], BF16)      # squared query tiles
            onek = const.tile([128, 1], BF16)
            oneq = const.tile([128, 1], BF16)
            ident = const.tile([128, 128], BF16)
            ct32 = const.tile([128, D], F32)
            cdump = const.tile([128, D], F32)
            csum = const.tile([128, 1], F32)
            cnorm = const.tile([128, 1], F32)
            rnorm = const.tile([128, 1], F32)
            cn32 = const.tile([128, D], F32)
            cnb = const.tile([128, D], BF16)
            cT = const.tile([128, KD, 128], F8)        # -cn^T tiles
            cT6 = const.tile([128, 128], BF16)         # center aug rows
            eps30 = const.tile([128, 1], F32)
            marg = const.tile([128, 1], F32)
            apc = const.tile([128, MQ, NQR], F32)
            anc = const.tile([128, MQ, NQR], F32)
            apmax = const.tile([128, MQ], F32)
            anmin = const.tile([128, MQ], F32)
            wmin = const.tile([128, MQ], F32)

            # ---------- sync engine: all load DMAs ----------
            nc.sync.dma_start(out=qT[:].rearrange("p s n -> p (s n)"),
                              in_=xq2T_h[:, :])
            for q in range(NQR):
                nc.sync.dma_start(
                    out=kT[q][:].rearrange("p s n -> p (s n)"),
                    in_=xT_h[:, q * KD * QW : (q + 1) * KD * QW],
                )
                if q == 0:
                    nc.sync.dma_start(out=qT6[:], in_=augq_h[:, :])
                    nc.sync.dma_start(out=cT6[:], in_=augc_h[:, :])
                    nc.sync.dma_start(out=ct32[0:P, :], in_=cen_h[:, :])
                    nc.sync.dma_start(out=kT6[:], in_=augk_h[:, :])

            # ---------- vector: init ----------
            nc.vector.memset(ct32[96:128, :], 0.0)
            nc.vector.memset(eps30[:], 1e-30)
            nc.vector.memset(marg[:], MARGIN)
            nc.vector.memset(onek[:], 1.0)
            nc.vector.memset(oneq[:], 0.25)            # undo the 2x query scale
            make_identity(nc, ident[:])

            # ---------- row-norm chain: squares -> ones-matmul -> aug rows --
            # query side first (gates center GEMM and all aug matmuls)
            nc.scalar.activation(out=qsq[:].rearrange("p s n -> p (s n)"),
                                 in_=qT[:].rearrange("p s n -> p (s n)"),
                                 func=ACTF.Square)
            qrow = psmall.tile([1, NQ], F32, tag="sq", name="qrow")
            for s in range(KD):
                nc.tensor.matmul(qrow[:], oneq[:], qsq[:, s, :],
                                 start=(s == 0), stop=(s == KD - 1))
            nc.scalar.activation(out=qT6[0:1, :], in_=qrow[:],
                                 func=ACTF.Copy)

            # center normalize chain (early: scalar+vector are idle here)
            nc.scalar.activation(out=cdump[:], in_=ct32[:], func=ACTF.Square,
                                 accum_out=csum[:])
            nc.scalar.activation(out=cnorm[:], in_=csum[:], func=ACTF.Sqrt,
                                 bias=eps30[:])
            nc.vector.reciprocal(rnorm[:], cnorm[:])
            nc.vector.tensor_scalar_mul(rnorm[:], rnorm[:], -1.0)
            nc.vector.tensor_scalar(out=cn32[:], in0=ct32[:],
                                    scalar1=rnorm[:, 0:1], scalar2=None,
                                    op0=ALU.mult)
            nc.vector.tensor_copy(cnb[:], cn32[:])

            # ---------- main GEMM: quarters x m-tiles (fp8 DoubleRow) -------
            for q in range(NQR):
                # squares + key row-norm rows for this quarter
                ksq = ksqp.tile([128, KD, QW], BF16, tag="ksq", name=f"ksq{q}")
                nc.scalar.activation(out=ksq[:].rearrange("p s n -> p (s n)"),
                                     in_=kT[q][:].rearrange("p s n -> p (s n)"),
                                     func=ACTF.Square)

                # d<6 for m0..m2, then their augs, then m3 (3 PSUM bufs)
                def emit_main(m, pt):
                    for t in range(KD // 2):
                        lhsT = qT[:, 2 * t : 2 * t + 2, m * 128 : (m + 1) * 128]
                        for jj in range(QW // 512):
                            nc.tensor.matmul(
                                pt[:, jj * 512 : (jj + 1) * 512], lhsT,
                                kT[q][:, 2 * t : 2 * t + 2,
                                      jj * 512 : (jj + 1) * 512],
                                start=(t == 0), stop=False, perf_mode=DR,
                            )

                def emit_aug(m, pt):
                    lhsT = qT6[:, m * 128 : (m + 1) * 128]
                    for jj in range(QW // 512):
                        nc.tensor.matmul(
                            pt[:, jj * 512 : (jj + 1) * 512], lhsT,
                            kT6[:, q * QW + jj * 512 : q * QW + (jj + 1) * 512],
                            start=False, stop=True,
                        )

                def emit_red(m, pt):
                    nc.vector.tensor_reduce(out=apc[:, m, q : q + 1],
                                            in_=pt[:], axis=AX.X, op=ALU.max)
                    nc.vector.tensor_reduce(out=anc[:, m, q : q + 1],
                                            in_=pt[:], axis=AX.X, op=ALU.min)

                pts = []
                for m in range(MQ - 1):
                    pt = pmain.tile([128, QW], F32, tag="mm", name=f"pt{q}_{m}")
                    emit_main(m, pt)
                    pts.append(pt)
                # key row-norm matmuls after d<6 (ksq lands during them)
                for jj in range(QW // 512):
                    krow = psmall.tile([1, 512], F32, tag="sq",
                                       name=f"krow{q}_{jj}")
                    for s in range(KD):
                        nc.tensor.matmul(
                            krow[:], onek[:],
                            ksq[:, s, jj * 512 : (jj + 1) * 512],
                            start=(s == 0), stop=(s == KD - 1))
                    nc.scalar.activation(
                        out=kT6[96:97, q * QW + jj * 512 : q * QW + (jj + 1) * 512],
                        in_=krow[:], func=ACTF.Copy)
                for m in range(MQ - 1):
                    emit_aug(m, pts[m])
                for m in range(MQ - 1):
                    emit_red(m, pts[m])
                m = MQ - 1
                pt = pmain.tile([128, QW], F32, tag="mm", name=f"pt{q}_{m}")
                emit_main(m, pt)
                emit_aug(m, pt)
                emit_red(m, pt)

                if q == 0:
                    # center GEMM slots into the post-q0 PE bubble
                    for s in range(KD):
                        pv = psmall.tile([128, 128], BF16, tag="sq",
                                         name=f"ctr{s}")
                        nc.tensor.transpose(pv[:],
                                            cnb[:, s * 128 : (s + 1) * 128],
                                            ident[:])
                        nc.vector.tensor_copy(cT[:, s, :], pv[:])
                    pc = pcp.tile([128, MQ, 128], F32, tag="pc", name="pc")
                    for m in range(MQ):
                        for t in range(KD // 2):
                            nc.tensor.matmul(pc[:, m, :],
                                             qT[:, 2 * t : 2 * t + 2,
                                                m * 128 : (m + 1) * 128],
                                             cT[:, 2 * t : 2 * t + 2, :],
                                             start=(t == 0), stop=False,
                                             perf_mode=DR)
                        nc.tensor.matmul(pc[:, m, :],
                                         qT6[:, m * 128 : (m + 1) * 128],
                                         cT6[:], start=False, stop=True)

            # ---------- finals ----------
            nc.vector.tensor_reduce(out=apmax[:], in_=apc[:], axis=AX.X, op=ALU.max)
            nc.vector.tensor_reduce(out=anmin[:], in_=anc[:], axis=AX.X, op=ALU.min)
            nc.vector.tensor_reduce(out=wmin[:], in_=pc[:], axis=AX.X, op=ALU.min)
            ap2 = const.tile([128, MQ], F32)
            nc.vector.tensor_scalar_add(ap2[:], apmax[:], -BIG)
            nc.vector.tensor_scalar_max(ap2[:], ap2[:], 1e-12)
            ap_d = const.tile([128, MQ], F32)
            nc.scalar.activation(out=ap_d[:], in_=ap2[:], func=ACTF.Sqrt)

            an2 = const.tile([128, MQ], F32)
            nc.vector.tensor_scalar_max(an2[:], anmin[:], 1e-12)
            an_d = const.tile([128, MQ], F32)
            nc.scalar.activation(out=an_d[:], in_=an2[:], func=ACTF.Sqrt)

            dc2 = const.tile([128, MQ], F32)
            nc.vector.tensor_scalar_max(dc2[:], wmin[:], 0.0)
            dc_d = const.tile([128, MQ], F32)
            nc.scalar.activation(out=dc_d[:], in_=dc2[:], func=ACTF.Sqrt)
            nc.vector.tensor_scalar_max(dc_d[:], dc_d[:], 1e-12)

            an_f = const.tile([128, MQ], F32)
            nc.vector.tensor_tensor(out=an_f[:], in0=an_d[:], in1=dc_d[:],
                                    op=ALU.min)
            diff = const.tile([128, MQ], F32)
            nc.vector.tensor_tensor(out=diff[:], in0=ap_d[:], in1=an_f[:],
                                    op=ALU.subtract)
            lvec = const.tile([128, MQ], F32)
            nc.scalar.activation(out=lvec[:], in_=diff[:], func=ACTF.Relu,
                                 bias=marg[:])
            lcol = const.tile([128, 1], F32)
            nc.vector.tensor_reduce(out=lcol[:], in_=lvec[:], axis=AX.X, op=ALU.add)
            lsum = const.tile([128, 1], F32)
            nc.gpsimd.partition_all_reduce(lsum[:], lcol[:], 128,
                                           bass_isa.ReduceOp.add)
            nc.sync.dma_start(out=loss_h[:], in_=lsum[0:1, 0:1])

    nc.finalize()
    return nc


def _get_nc():
    global _nc_cache
    if _nc_cache is None:
        _nc_cache = _build()
    return _nc_cache


def _in_maps(inputs, targets, center):
    x = np.asarray(inputs, dtype=np.float32)
    t = np.asarray(targets).astype(np.int64).reshape(-1)
    c = np.ascontiguousarray(np.asarray(center, dtype=np.float32))
    assert x.shape == (N, D) and t.shape == (N,) and c.shape == (P, D)

    xneg = (-x).astype(NPF8)                           # key values, fp8
    x2 = (2.0 * x).astype(NPF8)                        # query values, fp8
    # key side -X^T, quarter-blocked: [p, q*(KD*QW) + s*QW + j]
    xT = np.ascontiguousarray(
        xneg.T.reshape(KD, 128, NQR, QW).transpose(1, 2, 0, 3).reshape(128, KD * N)
    )

    # aug row map: row 0 = sq_i, row 96 = sq_j/csq,
    # classes 0..94 -> rows 1..95, classes 95..99 -> rows 97..101
    rows = np.where(t < 95, t + 1, t + 2)
    augk = np.zeros((128, N), dtype=NPBF16)
    augk[rows, np.arange(N)] = NPBF16(BIG)
    augk[0, :] = NPBF16(1.0)                           # sq_i coefficient

    # center aug rows: csq (=1) at row 96, huge for pad centers, sq_i coeff
    augc = np.zeros((128, 128), dtype=NPBF16)
    augc[96, 0:P] = NPBF16(1.0)
    augc[96, P:128] = NPBF16(1.0e6)
    augc[0, :] = NPBF16(1.0)

    maps = []
    for core in range(N_CORES):
        s = slice(core * NQ, (core + 1) * NQ)
        xq2T = np.ascontiguousarray(
            x2[s].T.reshape(KD, 128, NQ).transpose(1, 0, 2).reshape(128, KD * NQ)
        )
        augq = np.zeros((128, NQ), dtype=NPBF16)
        augq[rows[s], np.arange(NQ)] = NPBF16(1.0)
        augq[96, :] = NPBF16(1.0)                      # sq_j coefficient
        maps.append({
            "xT": xT,
            "xq2T": xq2T,
            "augk": augk,
            "augq": augq,
            "augc": augc,
            "center": c,
        })
    return maps


def run(inputs, targets, center, trace=False):
    nc = _get_nc()
    res = run_bass_kernel_spmd(
        nc, _in_maps(inputs, targets, center), list(range(N_CORES)), trace=trace
    )
    tot = sum(float(r["loss"][0, 0]) for r in res.results)
    loss = np.float32(tot / N)
    return np.asarray(loss), res


def kernel(inputs, targets, center):
    out, _ = run(inputs, targets, center)
    return out


# revision 37
# speedup vs baseline: 1.1001x; 1.1001x over previous
"""AugmentedTripletLoss Trainium2 kernel — 8-core SPMD, row-sharded.

Math (matches reference):
  d2[i,j] = sq_i + sq_j - 2*X@X.T
  ap_i    = sqrt(clip(max_{same class} d2, 1e-12))
  an_i    = min( sqrt(clip(min_{diff class} d2, 1e-12)),
                 clip(sqrt(clip(sq_i + csq_c - 2*x_i.cn_c, 0)), 1e-12) )
  loss    = mean(relu(1 + ap - an))

Strategy (per core, 512 query rows):
  Host marshals layouts only (transposes / sign scales / one-hot encodes —
  no FLOPs): keys as -X^T fp8 tiles (quarter-blocked for 12KB DMA rows),
  queries as 2*X_q^T fp8, one-hot class aug tiles in bf16.
  Main GEMM runs fp8 DoubleRow (2 contraction subtiles per matmul).
  Row norms are computed on-device from the same fp8 tiles: Scalar
  squares them into bf16, a ones-vector matmul row-reduces into a
  [1,512] PSUM row, and Scalar copies that into the bf16 aug rows
  (sq_j at aug row 96 against query coeff 1; sq_i at aug row 102
  against key coeff 1; BIG*onehot rows complete the aug tile).
  Each [128,1024] PSUM tile then holds u = d2 + BIG*[same class], so
  the masked max/min are plain DVE tensor_reduce passes. Work is
  quarter-pipelined over key columns. Centers: on-device normalize,
  negate, fp8 PE-transpose; csq/sq_i ride the aug rows.
  Final: per-core partial sum -> host gathers the 8 scalars, sums, /N.
"""
import os
import sys

for _p in ("/opt/trn_rl_repo", "/root/.axon_site"):
    if _p not in sys.path:
        sys.path.insert(0, _p)

import numpy as np
import ml_dtypes

import concourse.bass as bass
import concourse.bacc as bacc
import concourse.mybir as mybir
import concourse.bass_isa as bass_isa
from concourse.tile import TileContext
from concourse.masks import make_identity
from concourse.bass_utils import run_bass_kernel_spmd

F32 = mybir.dt.float32
BF16 = mybir.dt.bfloat16
F8 = mybir.dt.float8e4
ALU = mybir.AluOpType
ACTF = mybir.ActivationFunctionType
AX = mybir.AxisListType
DR = mybir.MatmulPerfMode.DoubleRow
NPBF16 = ml_dtypes.bfloat16
NPF8 = ml_dtypes.float8_e4m3

N_CORES = 8
N, D, P = 4096, 768, 100
NQ = N // N_CORES        # 512 query rows per core
MQ = NQ // 128           # 4 query m-tiles
KD = D // 128            # 6 contraction tiles
NQR = 4                  # key-column quarters
QW = N // NQR            # 1024 cols per quarter
BIG = 16384.0
MARGIN = 1.0

_nc_cache = None


def _build():
    nc = bacc.Bacc("TRN2", target_bir_lowering=False, num_devices=N_CORES)

    # xT: quarter-blocked -X^T fp8; col q*(KD*QW) + s*QW + j = -x[q*QW+j, 128s+p]
    xT_h = nc.declare_dram_parameter("xT", [128, KD * N], F8, isOutput=False)
    xq2T_h = nc.declare_dram_parameter("xq2T", [128, KD * NQ], F8, isOutput=False)
    augk_h = nc.declare_dram_parameter("augk", [128, N], BF16, isOutput=False)
    augq_h = nc.declare_dram_parameter("augq", [128, NQ], BF16, isOutput=False)
    augc_h = nc.declare_dram_parameter("augc", [128, 128], BF16, isOutput=False)
    cen_h = nc.declare_dram_parameter("center", [P, D], F32, isOutput=False)
    loss_h = nc.declare_dram_parameter("loss", [1, 1], F32, isOutput=True)

    with TileContext(nc) as tc:
        from contextlib import ExitStack

        with ExitStack() as ctx:
            const = ctx.enter_context(tc.tile_pool(name="const", bufs=1))
            ksqp = ctx.enter_context(tc.tile_pool(name="ksqp", bufs=2))
            pmain = ctx.enter_context(tc.tile_pool(name="pmain", bufs=3, space="PSUM"))
            psmall = ctx.enter_context(tc.tile_pool(name="psmall", bufs=1,
                                                    space="PSUM"))
            pcp = ctx.enter_context(tc.tile_pool(name="pcp", bufs=1,
                                                 space="PSUM"))

            # ---------- persistent tiles ----------
            kT = [const.tile([128, KD, QW], F8, name=f"kT{q}")
                  for q in range(NQR)]                 # -X^T keys, per quarter
            kT6 = const.tile([128, N], BF16)           # aug keys
            qT = const.tile([128, KD, NQ], F8)         # 2*X_q^T query tiles
            qT6 = const.tile([128, NQ], BF16)          # aug queries
            qsq = const.tile([128, KD, NQ], BF16)      # squared query tiles
            onek = const.tile([128, 1], BF16)
            oneq = const.tile([128, 1], BF16)
            ident = const.tile([128, 128], BF16)
            ct32 = const.tile([128, D], F32)
            cdump = const.tile([128, D], F32)
            csum = const.tile([128, 1], F32)
            cnorm = const.tile([128, 1], F32)
            rnorm = const.tile([128, 1], F32)
            cn32 = const.tile([128, D], F32)
            cnb = const.tile([128, D], BF16)
            cT = const.tile([128, KD, 128], F8)        # -cn^T tiles
            cT6 = const.tile([128, 128], BF16)         # center aug rows
            eps30 = const.tile([128, 1], F32)
            marg = const.tile([128, 1], F32)
            apc = const.tile([128, MQ, NQR], F32)
            anc = const.tile([128, MQ, NQR], F32)
            apmax = const.tile([128, MQ], F32)
            anmin = const.tile([128, MQ], F32)
            wmin = const.tile([128, MQ], F32)

            # ---------- sync engine: all load DMAs ----------
            for q in range(NQR):
                nc.sync.dma_start(
                    out=kT[q][:].rearrange("p s n -> p (s n)"),
                    in_=xT_h[:, q * KD * QW : (q + 1) * KD * QW],
                )
                if q == 0:
                    nc.sync.dma_start(out=qT[:].rearrange("p s n -> p (s n)"),
                                      in_=xq2T_h[:, :])
                    nc.sync.dma_start(out=qT6[:], in_=augq_h[:, :])
                    nc.sync.dma_start(out=cT6[:], in_=augc_h[:, :])
                    nc.sync.dma_start(out=ct32[0:P, :], in_=cen_h[:, :])
                    nc.sync.dma_start(out=kT6[:], in_=augk_h[:, :])

            # ---------- vector: init ----------
            nc.vector.memset(ct32[96:128, :], 0.0)
            nc.vector.memset(eps30[:], 1e-30)
            nc.vector.memset(marg[:], MARGIN)
            nc.vector.memset(onek[:], 1.0)
            nc.vector.memset(oneq[:], 0.25)            # undo the 2x query scale
            make_identity(nc, ident[:])

            # ---------- row-norm chain: squares -> ones-matmul -> aug rows --
            # query side first (gates center GEMM and all aug matmuls)
            nc.scalar.activation(out=qsq[:].rearrange("p s n -> p (s n)"),
                                 in_=qT[:].rearrange("p s n -> p (s n)"),
                                 func=ACTF.Square)
            qrow = psmall.tile([1, NQ], F32, tag="sq", name="qrow")
            for s in range(KD):
                nc.tensor.matmul(qrow[:], oneq[:], qsq[:, s, :],
                                 start=(s == 0), stop=(s == KD - 1))
            nc.vector.tensor_copy(qT6[0:1, :], qrow[:])

            # ---------- main GEMM: quarters x m-tiles (fp8 DoubleRow) -------
            for q in range(NQR):
                # squares for this quarter, split by jj half for latency
                ksq = ksqp.tile([128, KD, QW], BF16, tag="ksq", name=f"ksq{q}")
                for jj in range(QW // 512):
                    nc.scalar.activation(
                        out=ksq[:, :, jj * 512 : (jj + 1) * 512],
                        in_=kT[q][:, :, jj * 512 : (jj + 1) * 512],
                        func=ACTF.Square)

                # d<6 for m0..m2, then their augs, then m3 (3 PSUM bufs)
                def emit_main(m, pt):
                    for t in range(KD // 2):
                        lhsT = qT[:, 2 * t : 2 * t + 2, m * 128 : (m + 1) * 128]
                        for jj in range(QW // 512):
                            nc.tensor.matmul(
                                pt[:, jj * 512 : (jj + 1) * 512], lhsT,
                                kT[q][:, 2 * t : 2 * t + 2,
                                      jj * 512 : (jj + 1) * 512],
                                start=(t == 0), stop=False, perf_mode=DR,
                            )

                def emit_aug(m, pt):
                    lhsT = qT6[:, m * 128 : (m + 1) * 128]
                    for jj in range(QW // 512):
                        nc.tensor.matmul(
                            pt[:, jj * 512 : (jj + 1) * 512], lhsT,
                            kT6[:, q * QW + jj * 512 : q * QW + (jj + 1) * 512],
                            start=False, stop=True,
                        )

                def emit_red(m, pt):
                    nc.vector.tensor_reduce(out=apc[:, m, q : q + 1],
                                            in_=pt[:], axis=AX.X, op=ALU.max)
                    nc.vector.tensor_reduce(out=anc[:, m, q : q + 1],
                                            in_=pt[:], axis=AX.X, op=ALU.min)

                pts = []
                for m in range(MQ - 1):
                    pt = pmain.tile([128, QW], F32, tag="mm", name=f"pt{q}_{m}")
                    emit_main(m, pt)
                    pts.append(pt)
                # key row-norm matmuls after d<6 (ksq lands during them)
                for jj in range(QW // 512):
                    krow = psmall.tile([1, 512], F32, tag="sq",
                                       name=f"krow{q}_{jj}")
                    for s in range(KD):
                        nc.tensor.matmul(
                            krow[:], onek[:],
                            ksq[:, s, jj * 512 : (jj + 1) * 512],
                            start=(s == 0), stop=(s == KD - 1))
                    dst = kT6[96:97, q * QW + jj * 512 : q * QW + (jj + 1) * 512]
                    if q == 0:
                        nc.vector.tensor_copy(dst, krow[:])
                    else:
                        nc.scalar.activation(out=dst, in_=krow[:],
                                             func=ACTF.Copy)
                for m in range(MQ - 1):
                    emit_aug(m, pts[m])
                for m in range(MQ - 1):
                    emit_red(m, pts[m])
                m = MQ - 1
                pt = pmain.tile([128, QW], F32, tag="mm", name=f"pt{q}_{m}")
                emit_main(m, pt)
                emit_aug(m, pt)
                emit_red(m, pt)

                if q == 0:
                    # center normalize chain + GEMM slot into the post-q0
                    # bubble (scalar/vector/PE all have slack here)
                    nc.scalar.activation(out=cdump[:], in_=ct32[:],
                                         func=ACTF.Square, accum_out=csum[:])
                    nc.scalar.activation(out=cnorm[:], in_=csum[:],
                                         func=ACTF.Sqrt, bias=eps30[:])
                    nc.vector.reciprocal(rnorm[:], cnorm[:])
                    nc.vector.tensor_scalar_mul(rnorm[:], rnorm[:], -1.0)
                    nc.vector.tensor_scalar(out=cn32[:], in0=ct32[:],
                                            scalar1=rnorm[:, 0:1], scalar2=None,
                                            op0=ALU.mult)
                    nc.vector.tensor_copy(cnb[:], cn32[:])
                    for s in range(KD):
                        pv = psmall.tile([128, 128], BF16, tag="sq",
                                         name=f"ctr{s}")
                        nc.tensor.transpose(pv[:],
                                            cnb[:, s * 128 : (s + 1) * 128],
                                            ident[:])
                        nc.vector.tensor_copy(cT[:, s, :], pv[:])
                    pc = pcp.tile([128, MQ, 128], F32, tag="pc", name="pc")
                    for m in range(MQ):
                        for t in range(KD // 2):
                            nc.tensor.matmul(pc[:, m, :],
                                             qT[:, 2 * t : 2 * t + 2,
                                                m * 128 : (m + 1) * 128],
                                             cT[:, 2 * t : 2 * t + 2, :],
                                             start=(t == 0), stop=False,
                                             perf_mode=DR)
                        nc.tensor.matmul(pc[:, m, :],
                                         qT6[:, m * 128 : (m + 1) * 128],
                                         cT6[:], start=False, stop=True)

            # ---------- finals ----------
            nc.vector.tensor_reduce(out=apmax[:], in_=apc[:], axis=AX.X, op=ALU.max)
            nc.vector.tensor_reduce(out=anmin[:], in_=anc[:], axis=AX.X, op=ALU.min)
            nc.vector.tensor_reduce(out=wmin[:], in_=pc[:], axis=AX.X, op=ALU.min)
            ap2 = const.tile([128, MQ], F32)
            nc.vector.tensor_scalar_add(ap2[:], apmax[:], -BIG)
            nc.vector.tensor_scalar_max(ap2[:], ap2[:], 1e-12)
            ap_d = const.tile([128, MQ], F32)
            nc.scalar.activation(out=ap_d[:], in_=ap2[:], func=ACTF.Sqrt)

            an2 = const.tile([128, MQ], F32)
            nc.vector.tensor_scalar_max(an2[:], anmin[:], 1e-12)
            an_d = const.tile([128, MQ], F32)
            nc.scalar.activation(out=an_d[:], in_=an2[:], func=ACTF.Sqrt)

            dc2 = const.tile([128, MQ], F32)
            nc.vector.tensor_scalar_max(dc2[:], wmin[:], 0.0)
            dc_d = const.tile([128, MQ], F32)
            nc.scalar.activation(out=dc_d[:], in_=dc2[:], func=ACTF.Sqrt)
            nc.vector.tensor_scalar_max(dc_d[:], dc_d[:], 1e-12)

            an_f = const.tile([128, MQ], F32)
            nc.vector.tensor_tensor(out=an_f[:], in0=an_d[:], in1=dc_d[:],
                                    op=ALU.min)
            diff = const.tile([128, MQ], F32)
            nc.vector.tensor_tensor(out=diff[:], in0=ap_d[:], in1=an_f[:],
                                    op=ALU.subtract)
            lvec = const.tile([128, MQ], F32)
            nc.scalar.activation(out=lvec[:], in_=diff[:], func=ACTF.Relu,
                                 bias=marg[:])
            lcol = const.tile([128, 1], F32)
            nc.vector.tensor_reduce(out=lcol[:], in_=lvec[:], axis=AX.X, op=ALU.add)
            lsum = const.tile([128, 1], F32)
            nc.gpsimd.partition_all_reduce(lsum[:], lcol[:], 128,
                                           bass_isa.ReduceOp.add)
            nc.sync.dma_start(out=loss_h[:], in_=lsum[0:1, 0:1])

    nc.finalize()
    return nc


def _get_nc():
    global _nc_cache
    if _nc_cache is None:
        _nc_cache = _build()
    return _nc_cache


def _in_maps(inputs, targets, center):
    x = np.asarray(inputs, dtype=np.float32)
    t = np.asarray(targets).astype(np.int64).reshape(-1)
    c = np.ascontiguousarray(np.asarray(center, dtype=np.float32))
    assert x.shape == (N, D) and t.shape == (N,) and c.shape == (P, D)

    xneg = (-x).astype(NPF8)                           # key values, fp8
    x2 = (2.0 * x).astype(NPF8)                        # query values, fp8
    # key side -X^T, quarter-blocked: [p, q*(KD*QW) + s*QW + j]
    xT = np.ascontiguousarray(
        xneg.T.reshape(KD, 128, NQR, QW).transpose(1, 2, 0, 3).reshape(128, KD * N)
    )

    # aug row map: row 0 = sq_i, row 96 = sq_j/csq,
    # classes 0..94 -> rows 1..95, classes 95..99 -> rows 97..101
    rows = np.where(t < 95, t + 1, t + 2)
    augk = np.zeros((128, N), dtype=NPBF16)
    augk[rows, np.arange(N)] = NPBF16(BIG)
    augk[0, :] = NPBF16(1.0)                           # sq_i coefficient

    # center aug rows: csq (=1) at row 96, huge for pad centers, sq_i coeff
    augc = np.zeros((128, 128), dtype=NPBF16)
    augc[96, 0:P] = NPBF16(1.0)
    augc[96, P:128] = NPBF16(1.0e6)
    augc[0, :] = NPBF16(1.0)

    maps = []
    for core in range(N_CORES):
        s = slice(core * NQ, (core + 1) * NQ)
        xq2T = np.ascontiguousarray(
            x2[s].T.reshape(KD, 128, NQ).transpose(1, 0, 2).reshape(128, KD * NQ)
        )
        augq = np.zeros((128, NQ), dtype=NPBF16)
        augq[rows[s], np.arange(NQ)] = NPBF16(1.0)
        augq[96, :] = NPBF16(1.0)                      # sq_j coefficient
        maps.append({
            "xT": xT,
            "xq2T": xq2T,
            "augk": augk,
            "augq": augq,
            "augc": augc,
            "center": c,
        })
    return maps


def run(inputs, targets, center, trace=False):
    nc = _get_nc()
    res = run_bass_kernel_spmd(
        nc, _in_maps(inputs, targets, center), list(range(N_CORES)), trace=trace
    )
    tot = sum(float(r["loss"][0, 0]) for r in res.results)
    loss = np.float32(tot / N)
    return np.asarray(loss), res


def kernel(inputs, targets, center):
    out, _ = run(inputs, targets, center)
    return out


# revision 39
# speedup vs baseline: 1.1456x; 1.0414x over previous
"""AugmentedTripletLoss Trainium2 kernel — 8-core SPMD, row-sharded.

Math (matches reference):
  d2[i,j] = sq_i + sq_j - 2*X@X.T
  ap_i    = sqrt(clip(max_{same class} d2, 1e-12))
  an_i    = min( sqrt(clip(min_{diff class} d2, 1e-12)),
                 clip(sqrt(clip(sq_i + csq_c - 2*x_i.cn_c, 0)), 1e-12) )
  loss    = mean(relu(1 + ap - an))

Strategy (per core, 512 query rows):
  Host marshals layouts only (transposes / sign scales / one-hot encodes —
  no FLOPs): keys as -X^T fp8 tiles (quarter-blocked for 12KB DMA rows),
  queries as 2*X_q^T fp8, one-hot class aug tiles in bf16.
  Main GEMM runs fp8 DoubleRow (2 contraction subtiles per matmul).
  Row norms are computed on-device from the same fp8 tiles: Scalar
  squares them into bf16, a ones-vector matmul row-reduces into a
  [1,512] PSUM row, and Scalar copies that into the bf16 aug rows
  (sq_j at aug row 96 against query coeff 1; sq_i at aug row 102
  against key coeff 1; BIG*onehot rows complete the aug tile).
  Each [128,1024] PSUM tile then holds u = d2 + BIG*[same class], so
  the masked max/min are plain DVE tensor_reduce passes. Work is
  quarter-pipelined over key columns. Centers: on-device normalize,
  negate, fp8 PE-transpose; csq/sq_i ride the aug rows.
  Final: per-core partial sum -> host gathers the 8 scalars, sums, /N.
"""
import os
import sys

for _p in ("/opt/trn_rl_repo", "/root/.axon_site"):
    if _p not in sys.path:
        sys.path.insert(0, _p)

import numpy as np
import ml_dtypes

import concourse.bass as bass
import concourse.bacc as bacc
import concourse.mybir as mybir
import concourse.bass_isa as bass_isa
from concourse.tile import TileContext
from concourse.masks import make_identity
from concourse.bass_utils import run_bass_kernel_spmd

F32 = mybir.dt.float32
BF16 = mybir.dt.bfloat16
F8 = mybir.dt.float8e4
ALU = mybir.AluOpType
ACTF = mybir.ActivationFunctionType
AX = mybir.AxisListType
DR = mybir.MatmulPerfMode.DoubleRow
NPBF16 = ml_dtypes.bfloat16
NPF8 = ml_dtypes.float8_e4m3

N_CORES = 8
N, D, P = 4096, 768, 100
NQ = N // N_CORES        # 512 query rows per core
MQ = NQ // 128           # 4 query m-tiles
KD = D // 128            # 6 contraction tiles
NQR = 4                  # key-column quarters
QW = N // NQR            # 1024 cols per quarter
BIG = 16384.0
MARGIN = 1.0

_nc_cache = None


def _build():
    nc = bacc.Bacc("TRN2", target_bir_lowering=False, num_devices=N_CORES)

    # xT: quarter-blocked -X^T fp8; col q*(KD*QW) + s*QW + j = -x[q*QW+j, 128s+p]
    xT_h = nc.declare_dram_parameter("xT", [128, KD * N], F8, isOutput=False)
    xq2T_h = nc.declare_dram_parameter("xq2T", [128, KD * NQ], F8, isOutput=False)
    augk_h = nc.declare_dram_parameter("augk", [128, N], BF16, isOutput=False)
    augq_h = nc.declare_dram_parameter("augq", [128, NQ], BF16, isOutput=False)
    augc_h = nc.declare_dram_parameter("augc", [128, 128], BF16, isOutput=False)
    cen_h = nc.declare_dram_parameter("center", [P, D], F32, isOutput=False)
    loss_h = nc.declare_dram_parameter("loss", [1, 1], F32, isOutput=True)

    with TileContext(nc) as tc:
        from contextlib import ExitStack

        with ExitStack() as ctx:
            const = ctx.enter_context(tc.tile_pool(name="const", bufs=1))
            ksqp = ctx.enter_context(tc.tile_pool(name="ksqp", bufs=2))
            pmain = ctx.enter_context(tc.tile_pool(name="pmain", bufs=3, space="PSUM"))
            psmall = ctx.enter_context(tc.tile_pool(name="psmall", bufs=1,
                                                    space="PSUM"))
            pcp = ctx.enter_context(tc.tile_pool(name="pcp", bufs=1,
                                                 space="PSUM"))

            # ---------- persistent tiles ----------
            kT = [const.tile([128, KD, QW], F8, name=f"kT{q}")
                  for q in range(NQR)]                 # -X^T keys, per quarter
            kT6 = const.tile([128, N], BF16)           # aug keys
            qT = const.tile([128, KD, NQ], F8)         # 2*X_q^T query tiles
            qT6 = const.tile([128, NQ], BF16)          # aug queries
            qsq = const.tile([128, KD, NQ], BF16)      # squared query tiles
            onek = const.tile([128, 1], BF16)
            oneq = const.tile([128, 1], BF16)
            ident = const.tile([128, 128], BF16)
            ct32 = const.tile([128, D], F32)
            cdump = const.tile([128, D], F32)
            csum = const.tile([128, 1], F32)
            cnorm = const.tile([128, 1], F32)
            rnorm = const.tile([128, 1], F32)
            cn32 = const.tile([128, D], F32)
            cnb = const.tile([128, D], BF16)
            cT = const.tile([128, KD, 128], F8)        # -cn^T tiles
            cT6 = const.tile([128, 128], BF16)         # center aug rows
            eps30 = const.tile([128, 1], F32)
            marg = const.tile([128, 1], F32)
            apc = const.tile([128, MQ, NQR], F32)
            anc = const.tile([128, MQ, NQR], F32)
            apmax = const.tile([128, MQ], F32)
            anmin = const.tile([128, MQ], F32)
            wmin = const.tile([128, MQ], F32)

            # ---------- sync engine: all load DMAs ----------
            nc.sync.dma_start(out=qT[:].rearrange("p s n -> p (s n)"),
                              in_=xq2T_h[:, :])
            for q in range(NQR):
                nc.sync.dma_start(
                    out=kT[q][:].rearrange("p s n -> p (s n)"),
                    in_=xT_h[:, q * KD * QW : (q + 1) * KD * QW],
                )
                if q == 0:
                    nc.sync.dma_start(out=kT6[:], in_=augk_h[:, :])
                    nc.sync.dma_start(out=qT6[:], in_=augq_h[:, :])
                    nc.sync.dma_start(out=cT6[:], in_=augc_h[:, :])
                    nc.sync.dma_start(out=ct32[0:P, :], in_=cen_h[:, :])

            # ---------- vector: init ----------
            nc.vector.memset(ct32[96:128, :], 0.0)
            nc.vector.memset(eps30[:], 1e-30)
            nc.vector.memset(marg[:], MARGIN)
            nc.vector.memset(onek[:], 1.0)
            nc.vector.memset(oneq[:], 0.25)            # undo the 2x query scale
            make_identity(nc, ident[:])

            # ---------- row-norm chain: squares -> ones-matmul -> aug rows --
            # query side first (gates center GEMM and all aug matmuls)
            nc.scalar.activation(out=qsq[:].rearrange("p s n -> p (s n)"),
                                 in_=qT[:].rearrange("p s n -> p (s n)"),
                                 func=ACTF.Square)
            qrow = psmall.tile([1, NQ], F32, tag="sq", name="qrow")
            for s in range(KD):
                nc.tensor.matmul(qrow[:], oneq[:], qsq[:, s, :],
                                 start=(s == 0), stop=(s == KD - 1))
            nc.vector.tensor_copy(qT6[0:1, :], qrow[:])

            # ---------- main GEMM: quarters x m-tiles (fp8 DoubleRow) -------
            for q in range(NQR):
                # squares for this quarter, split by jj half for latency
                ksq = ksqp.tile([128, KD, QW], BF16, tag="ksq", name=f"ksq{q}")
                for jj in range(QW // 512):
                    nc.scalar.activation(
                        out=ksq[:, :, jj * 512 : (jj + 1) * 512],
                        in_=kT[q][:, :, jj * 512 : (jj + 1) * 512],
                        func=ACTF.Square)

                # d<6 for m0..m2, then their augs, then m3 (3 PSUM bufs)
                def emit_main(m, pt):
                    for t in range(KD // 2):
                        lhsT = qT[:, 2 * t : 2 * t + 2, m * 128 : (m + 1) * 128]
                        for jj in range(QW // 512):
                            nc.tensor.matmul(
                                pt[:, jj * 512 : (jj + 1) * 512], lhsT,
                                kT[q][:, 2 * t : 2 * t + 2,
                                      jj * 512 : (jj + 1) * 512],
                                start=(t == 0), stop=False, perf_mode=DR,
                            )

                def emit_aug(m, pt):
                    lhsT = qT6[:, m * 128 : (m + 1) * 128]
                    for jj in range(QW // 512):
                        nc.tensor.matmul(
                            pt[:, jj * 512 : (jj + 1) * 512], lhsT,
                            kT6[:, q * QW + jj * 512 : q * QW + (jj + 1) * 512],
                            start=False, stop=True,
                        )

                def emit_red(m, pt):
                    nc.vector.tensor_reduce(out=apc[:, m, q : q + 1],
                                            in_=pt[:], axis=AX.X, op=ALU.max)
                    nc.vector.tensor_reduce(out=anc[:, m, q : q + 1],
                                            in_=pt[:], axis=AX.X, op=ALU.min)

                pts = []
                for m in range(MQ - 1):
                    pt = pmain.tile([128, QW], F32, tag="mm", name=f"pt{q}_{m}")
                    emit_main(m, pt)
                    pts.append(pt)
                # key row-norm matmuls after d<6 (ksq lands during them)
                for jj in range(QW // 512):
                    krow = psmall.tile([1, 512], F32, tag="sq",
                                       name=f"krow{q}_{jj}")
                    for s in range(KD):
                        nc.tensor.matmul(
                            krow[:], onek[:],
                            ksq[:, s, jj * 512 : (jj + 1) * 512],
                            start=(s == 0), stop=(s == KD - 1))
                    dst = kT6[96:97, q * QW + jj * 512 : q * QW + (jj + 1) * 512]
                    if q == 0:
                        nc.vector.tensor_copy(dst, krow[:])
                    else:
                        nc.scalar.activation(out=dst, in_=krow[:],
                                             func=ACTF.Copy)
                for m in range(MQ - 1):
                    emit_aug(m, pts[m])
                for m in range(MQ - 1):
                    emit_red(m, pts[m])
                m = MQ - 1
                pt = pmain.tile([128, QW], F32, tag="mm", name=f"pt{q}_{m}")
                emit_main(m, pt)
                emit_aug(m, pt)
                emit_red(m, pt)

                if q == 0:
                    # center normalize chain + GEMM slot into the post-q0
                    # bubble (scalar/vector/PE all have slack here)
                    nc.scalar.activation(out=cdump[:], in_=ct32[:],
                                         func=ACTF.Square, accum_out=csum[:])
                    nc.scalar.activation(out=cnorm[:], in_=csum[:],
                                         func=ACTF.Sqrt, bias=eps30[:])
                    nc.vector.reciprocal(rnorm[:], cnorm[:])
                    nc.vector.tensor_scalar_mul(rnorm[:], rnorm[:], -1.0)
                    nc.vector.tensor_scalar(out=cn32[:], in0=ct32[:],
                                            scalar1=rnorm[:, 0:1], scalar2=None,
                                            op0=ALU.mult)
                    nc.vector.tensor_copy(cnb[:], cn32[:])
                    for s in range(KD):
                        pv = pcp.tile([128, 128], BF16, tag="pc",
                                      name=f"ctr{s}")
                        nc.tensor.transpose(pv[:],
                                            cnb[:, s * 128 : (s + 1) * 128],
                                            ident[:])
                        nc.vector.tensor_copy(cT[:, s, :], pv[:])
                    pc = pcp.tile([128, MQ, 128], F32, tag="pc", name="pc")
                    for m in range(MQ):
                        for t in range(KD // 2):
                            nc.tensor.matmul(pc[:, m, :],
                                             qT[:, 2 * t : 2 * t + 2,
                                                m * 128 : (m + 1) * 128],
                                             cT[:, 2 * t : 2 * t + 2, :],
                                             start=(t == 0), stop=False,
                                             perf_mode=DR)
                        nc.tensor.matmul(pc[:, m, :],
                                         qT6[:, m * 128 : (m + 1) * 128],
                                         cT6[:], start=False, stop=True)

            # ---------- finals ----------
            nc.vector.tensor_reduce(out=apmax[:], in_=apc[:], axis=AX.X, op=ALU.max)
            nc.vector.tensor_reduce(out=anmin[:], in_=anc[:], axis=AX.X, op=ALU.min)
            nc.vector.tensor_reduce(out=wmin[:], in_=pc[:], axis=AX.X, op=ALU.min)
            ap2 = const.tile([128, MQ], F32)
            nc.vector.tensor_scalar_add(ap2[:], apmax[:], -BIG)
            nc.vector.tensor_scalar_max(ap2[:], ap2[:], 1e-12)
            ap_d = const.tile([128, MQ], F32)
            nc.scalar.activation(out=ap_d[:], in_=ap2[:], func=ACTF.Sqrt)

            an2 = const.tile([128, MQ], F32)
            nc.vector.tensor_scalar_max(an2[:], anmin[:], 1e-12)
            an_d = const.tile([128, MQ], F32)
            nc.scalar.activation(out=an_d[:], in_=an2[:], func=ACTF.Sqrt)

            dc2 = const.tile([128, MQ], F32)
            nc.vector.tensor_scalar_max(dc2[:], wmin[:], 0.0)
            dc_d = const.tile([128, MQ], F32)
            nc.scalar.activation(out=dc_d[:], in_=dc2[:], func=ACTF.Sqrt)
            nc.vector.tensor_scalar_max(dc_d[:], dc_d[:], 1e-12)

            an_f = const.tile([128, MQ], F32)
            nc.vector.tensor_tensor(out=an_f[:], in0=an_d[:], in1=dc_d[:],
                                    op=ALU.min)
            diff = const.tile([128, MQ], F32)
            nc.vector.tensor_tensor(out=diff[:], in0=ap_d[:], in1=an_f[:],
                                    op=ALU.subtract)
            lvec = const.tile([128, MQ], F32)
            nc.scalar.activation(out=lvec[:], in_=diff[:], func=ACTF.Relu,
                                 bias=marg[:])
            lcol = const.tile([128, 1], F32)
            nc.vector.tensor_reduce(out=lcol[:], in_=lvec[:], axis=AX.X, op=ALU.add)
            lsum = const.tile([128, 1], F32)
            nc.gpsimd.partition_all_reduce(lsum[:], lcol[:], 128,
                                           bass_isa.ReduceOp.add)
            nc.sync.dma_start(out=loss_h[:], in_=lsum[0:1, 0:1])

    nc.finalize()
    return nc


def _get_nc():
    global _nc_cache
    if _nc_cache is None:
        _nc_cache = _build()
    return _nc_cache


def _in_maps(inputs, targets, center):
    x = np.asarray(inputs, dtype=np.float32)
    t = np.asarray(targets).astype(np.int64).reshape(-1)
    c = np.ascontiguousarray(np.asarray(center, dtype=np.float32))
    assert x.shape == (N, D) and t.shape == (N,) and c.shape == (P, D)

    xneg = (-x).astype(NPF8)                           # key values, fp8
    x2 = (2.0 * x).astype(NPF8)                        # query values, fp8
    # key side -X^T, quarter-blocked: [p, q*(KD*QW) + s*QW + j]
    xT = np.ascontiguousarray(
        xneg.T.reshape(KD, 128, NQR, QW).transpose(1, 2, 0, 3).reshape(128, KD * N)
    )

    # aug row map: row 0 = sq_i, row 96 = sq_j/csq,
    # classes 0..94 -> rows 1..95, classes 95..99 -> rows 97..101
    rows = np.where(t < 95, t + 1, t + 2)
    augk = np.zeros((128, N), dtype=NPBF16)
    augk[rows, np.arange(N)] = NPBF16(BIG)
    augk[0, :] = NPBF16(1.0)                           # sq_i coefficient

    # center aug rows: csq (=1) at row 96, huge for pad centers, sq_i coeff
    augc = np.zeros((128, 128), dtype=NPBF16)
    augc[96, 0:P] = NPBF16(1.0)
    augc[96, P:128] = NPBF16(1.0e6)
    augc[0, :] = NPBF16(1.0)

    maps = []
    for core in range(N_CORES):
        s = slice(core * NQ, (core + 1) * NQ)
        xq2T = np.ascontiguousarray(
            x2[s].T.reshape(KD, 128, NQ).transpose(1, 0, 2).reshape(128, KD * NQ)
        )
        augq = np.zeros((128, NQ), dtype=NPBF16)
        augq[rows[s], np.arange(NQ)] = NPBF16(1.0)
        augq[96, :] = NPBF16(1.0)                      # sq_j coefficient
        maps.append({
            "xT": xT,
            "xq2T": xq2T,
            "augk": augk,
            "augq": augq,
            "augc": augc,
            "center": c,
        })
    return maps


def run(inputs, targets, center, trace=False):
    nc = _get_nc()
    res = run_bass_kernel_spmd(
        nc, _in_maps(inputs, targets, center), list(range(N_CORES)), trace=trace
    )
    tot = sum(float(r["loss"][0, 0]) for r in res.results)
    loss = np.float32(tot / N)
    return np.asarray(loss), res


def kernel(inputs, targets, center):
    out, _ = run(inputs, targets, center)
    return out


# revision 53
# speedup vs baseline: 1.2167x; 1.0621x over previous
"""AugmentedTripletLoss Trainium2 kernel — 8-core SPMD, row-sharded.

Math (matches reference):
  d2[i,j] = sq_i + sq_j - 2*X@X.T
  ap_i    = sqrt(clip(max_{same class} d2, 1e-12))
  an_i    = min( sqrt(clip(min_{diff class} d2, 1e-12)),
                 clip(sqrt(clip(sq_i + csq_c - 2*x_i.cn_c, 0)), 1e-12) )
  loss    = mean(relu(1 + ap - an))

Strategy (per core, 512 query rows):
  Host marshals layouts only (transposes / sign scales / one-hot encodes —
  no FLOPs): keys as -X^T fp8 tiles (quarter-blocked for 12KB DMA rows),
  queries as 2*X_q^T fp8, one-hot class aug tiles in bf16.
  Main GEMM runs fp8 DoubleRow (2 contraction subtiles per matmul).
  Row norms are computed on-device from the same fp8 tiles: Scalar
  squares them into bf16, a ones-vector matmul row-reduces into a
  [1,512] PSUM row, and Scalar copies that into the bf16 aug rows
  (sq_j at aug row 96 against query coeff 1; sq_i at aug row 102
  against key coeff 1; BIG*onehot rows complete the aug tile).
  Each [128,1024] PSUM tile then holds u = d2 + BIG*[same class], so
  the masked max/min are plain DVE tensor_reduce passes. Work is
  quarter-pipelined over key columns. Centers: on-device normalize,
  negate, fp8 PE-transpose; csq/sq_i ride the aug rows.
  Final: per-core partial sum -> host gathers the 8 scalars, sums, /N.
"""
import os
import sys

for _p in ("/opt/trn_rl_repo", "/root/.axon_site"):
    if _p not in sys.path:
        sys.path.insert(0, _p)

import numpy as np
import ml_dtypes

import concourse.bass as bass
import concourse.bacc as bacc
import concourse.mybir as mybir
import concourse.bass_isa as bass_isa
from concourse.tile import TileContext
from concourse.masks import make_identity
from concourse.bass_utils import run_bass_kernel_spmd
from concourse.dve_ops import TENSOR_MASK_REDUCE as TMR

F32 = mybir.dt.float32
BF16 = mybir.dt.bfloat16
F8 = mybir.dt.float8e4
ALU = mybir.AluOpType
ACTF = mybir.ActivationFunctionType
AX = mybir.AxisListType
DR = mybir.MatmulPerfMode.DoubleRow
NPBF16 = ml_dtypes.bfloat16
NPF8 = ml_dtypes.float8_e4m3

N_CORES = 8
N, D, P = 4096, 768, 100
NQ = N // N_CORES        # 512 query rows per core
MQ = NQ // 128           # 4 query m-tiles
KD = D // 128            # 6 contraction tiles
NQR = 4                  # key-column quarters
QW = N // NQR            # 1024 cols per quarter
BIG = 16384.0
MARGIN = 1.0

_nc_cache = None


def _build():
    nc = bacc.Bacc("TRN2", target_bir_lowering=False, num_devices=N_CORES)

    # xT: quarter-blocked -X^T fp8; col q*(KD*QW) + s*QW + j = -x[q*QW+j, 128s+p]
    xT_h = nc.declare_dram_parameter("xT", [128, KD * N], F8, isOutput=False)
    xq2T_h = nc.declare_dram_parameter("xq2T", [128, KD * NQ], F8, isOutput=False)
    augk_h = nc.declare_dram_parameter("augk", [128, N], BF16, isOutput=False)
    augq_h = nc.declare_dram_parameter("augq", [128, NQ], BF16, isOutput=False)
    augc_h = nc.declare_dram_parameter("augc", [128, 128], BF16, isOutput=False)
    # per-row same-class window bounds in rotated key coords:
    # [p, 4m+{0,1,2,3}] = lo/hi within cols 0:1024, lo/hi within cols 1024:1536
    wb_h = nc.declare_dram_parameter("wb", [128, MQ * 4], F32, isOutput=False)
    cen_h = nc.declare_dram_parameter("center", [P, D], F32, isOutput=False)
    loss_h = nc.declare_dram_parameter("loss", [1, 1], F32, isOutput=True)
    dbg_h = (nc.declare_dram_parameter("dbg", [128, 8 * MQ], F32, isOutput=True)
             if os.environ.get("KDBG") == "1" else None)

    with TileContext(nc) as tc:
        from contextlib import ExitStack

        with ExitStack() as ctx:
            const = ctx.enter_context(tc.tile_pool(name="const", bufs=1))
            ksqp = ctx.enter_context(tc.tile_pool(name="ksqp", bufs=2))
            pmain = ctx.enter_context(tc.tile_pool(name="pmain", bufs=3, space="PSUM"))
            psmall = ctx.enter_context(tc.tile_pool(name="psmall", bufs=1,
                                                    space="PSUM"))
            pcp = ctx.enter_context(tc.tile_pool(name="pcp", bufs=1,
                                                 space="PSUM"))

            # ---------- persistent tiles ----------
            kT = [const.tile([128, KD, QW], F8, name=f"kT{q}")
                  for q in range(NQR)]                 # -X^T keys, per quarter
            kT6 = const.tile([128, N], BF16)           # aug keys
            qT = const.tile([128, KD, NQ], F8)         # 2*X_q^T query tiles
            qT6 = const.tile([128, NQ], BF16)          # aug queries
            qsq = const.tile([128, KD, NQ], BF16)      # squared query tiles
            onek = const.tile([128, 1], BF16)
            oneq = const.tile([128, 1], BF16)
            ident = const.tile([128, 128], BF16)
            ct32 = const.tile([128, D], F32)
            cdump = const.tile([128, D], F32)
            csum = const.tile([128, 1], F32)
            cnorm = const.tile([128, 1], F32)
            rnorm = const.tile([128, 1], F32)
            cn32 = const.tile([128, D], F32)
            cnb = const.tile([128, D], BF16)
            cT = const.tile([128, KD, 128], F8)        # -cn^T tiles
            cT6 = const.tile([128, 128], BF16)         # center aug rows
            eps30 = const.tile([128, 1], F32)
            marg = const.tile([128, 1], F32)
            anc = const.tile([128, MQ, NQR], F32)
            anmin = const.tile([128, MQ], F32)
            wmin = const.tile([128, MQ], F32)
            wb = const.tile([128, MQ * 4], F32)
            apw0 = const.tile([128, MQ], F32)
            apw1 = const.tile([128, MQ], F32)
            apw = const.tile([128, MQ], F32)
            mscr = const.tile([128, QW], F32)           # mask-reduce out dump

            # ---------- sync engine: all load DMAs ----------
            nc.sync.dma_start(out=qT[:].rearrange("p s n -> p (s n)"),
                              in_=xq2T_h[:, :])
            for q in range(NQR):
                nc.sync.dma_start(
                    out=kT[q][:].rearrange("p s n -> p (s n)"),
                    in_=xT_h[:, q * KD * QW : (q + 1) * KD * QW],
                )
                if q == 0:
                    nc.sync.dma_start(out=kT6[:], in_=augk_h[:, :])
                    nc.sync.dma_start(out=qT6[:], in_=augq_h[:, :])
                    nc.sync.dma_start(out=cT6[:], in_=augc_h[:, :])
                    nc.sync.dma_start(out=ct32[0:P, :], in_=cen_h[:, :])
                    nc.sync.dma_start(out=wb[:], in_=wb_h[:, :])

            # ---------- vector: init ----------
            nc.vector.memset(ct32[96:128, :], 0.0)
            nc.vector.memset(eps30[:], 1e-30)
            nc.vector.memset(marg[:], MARGIN)
            nc.vector.memset(onek[:], 1.0)
            nc.vector.memset(oneq[:], 0.25)            # undo the 2x query scale
            make_identity(nc, ident[:])

            # ---------- row-norm chain: squares -> ones-matmul -> aug rows --
            # query side first (gates center GEMM and all aug matmuls)
            nc.scalar.activation(out=qsq[:].rearrange("p s n -> p (s n)"),
                                 in_=qT[:].rearrange("p s n -> p (s n)"),
                                 func=ACTF.Square)
            qrow = psmall.tile([1, NQ], F32, tag="sq", name="qrow")
            for s in range(KD):
                nc.tensor.matmul(qrow[:], oneq[:], qsq[:, s, :],
                                 start=(s == 0), stop=(s == KD - 1))
            nc.vector.tensor_copy(qT6[0:1, :], qrow[:])

            # ---------- main GEMM: quarters x m-tiles (fp8 DoubleRow) -------
            for q in range(NQR):
                # squares for this quarter, split by jj half for latency
                ksq = ksqp.tile([128, KD, QW], BF16, tag="ksq", name=f"ksq{q}")
                for jj in range(QW // 512):
                    nc.scalar.activation(
                        out=ksq[:, :, jj * 512 : (jj + 1) * 512],
                        in_=kT[q][:, :, jj * 512 : (jj + 1) * 512],
                        func=ACTF.Square)

                # d<6 for m0..m2, then their augs, then m3 (3 PSUM bufs)
                def emit_main(m, pt):
                    for t in range(KD // 2):
                        lhsT = qT[:, 2 * t : 2 * t + 2, m * 128 : (m + 1) * 128]
                        for jj in range(QW // 512):
                            nc.tensor.matmul(
                                pt[:, jj * 512 : (jj + 1) * 512], lhsT,
                                kT[q][:, 2 * t : 2 * t + 2,
                                      jj * 512 : (jj + 1) * 512],
                                start=(t == 0), stop=False, perf_mode=DR,
                            )

                def emit_aug(m, pt):
                    lhsT = qT6[:, m * 128 : (m + 1) * 128]
                    for jj in range(QW // 512):
                        nc.tensor.matmul(
                            pt[:, jj * 512 : (jj + 1) * 512], lhsT,
                            kT6[:, q * QW + jj * 512 : q * QW + (jj + 1) * 512],
                            start=False, stop=True,
                        )

                def emit_red(m, pt):
                    nc.vector.tensor_reduce(out=anc[:, m, q : q + 1],
                                            in_=pt[:], axis=AX.X, op=ALU.min)
                    # same-class max: rotated layout puts every query's class
                    # window inside key cols [0, 1536) = q0 + first half of q1.
                    # Table-based custom DVE op; seed must be an immediate
                    # (a per-partition s1 AP wedges the exec unit), so the two
                    # chunks accumulate separately and finals max them.
                    if q == 0:
                        nc.vector._custom_dve(
                            TMR, out=mscr[:], in0=pt[:],
                            in1=wb[:, 4 * m + 1 : 4 * m + 2],
                            s0=wb[:, 4 * m : 4 * m + 1],
                            s1=-3.0e38, imm2=1.0,
                            accum_out=apw0[:, m : m + 1],
                        )
                    elif q == 1:
                        nc.vector._custom_dve(
                            TMR, out=mscr[:, 0:512], in0=pt[:, 0:512],
                            in1=wb[:, 4 * m + 3 : 4 * m + 4],
                            s0=wb[:, 4 * m + 2 : 4 * m + 3],
                            s1=-3.0e38, imm2=1.0,
                            accum_out=apw1[:, m : m + 1],
                        )

                pts = []
                for m in range(MQ - 1):
                    pt = pmain.tile([128, QW], F32, tag="mm", name=f"pt{q}_{m}")
                    emit_main(m, pt)
                    pts.append(pt)
                # key row-norm matmuls after d<6 (ksq lands during them)
                for jj in range(QW // 512):
                    krow = psmall.tile([1, 512], F32, tag="sq",
                                       name=f"krow{q}_{jj}")
                    for s in range(KD):
                        nc.tensor.matmul(
                            krow[:], onek[:],
                            ksq[:, s, jj * 512 : (jj + 1) * 512],
                            start=(s == 0), stop=(s == KD - 1))
                    dst = kT6[96:97, q * QW + jj * 512 : q * QW + (jj + 1) * 512]
                    if q == 0:
                        nc.vector.tensor_copy(dst, krow[:])
                    else:
                        nc.scalar.activation(out=dst, in_=krow[:],
                                             func=ACTF.Copy)
                for m in range(MQ - 1):
                    emit_aug(m, pts[m])
                for m in range(MQ - 1):
                    emit_red(m, pts[m])
                m = MQ - 1
                pt = pmain.tile([128, QW], F32, tag="mm", name=f"pt{q}_{m}")
                emit_main(m, pt)
                emit_aug(m, pt)
                emit_red(m, pt)

                if q == 0:
                    # center normalize chain + GEMM slot into the post-q0
                    # bubble (scalar/vector/PE all have slack here)
                    nc.scalar.activation(out=cdump[:], in_=ct32[:],
                                         func=ACTF.Square, accum_out=csum[:])
                    nc.scalar.activation(out=cnorm[:], in_=csum[:],
                                         func=ACTF.Sqrt, bias=eps30[:])
                    nc.vector.reciprocal(rnorm[:], cnorm[:])
                    nc.vector.tensor_scalar_mul(rnorm[:], rnorm[:], -1.0)
                    nc.vector.tensor_scalar(out=cn32[:], in0=ct32[:],
                                            scalar1=rnorm[:, 0:1], scalar2=None,
                                            op0=ALU.mult)
                    nc.vector.tensor_copy(cnb[:], cn32[:])
                    for s in range(KD):
                        pv = pcp.tile([128, 128], BF16, tag="pc",
                                      name=f"ctr{s}")
                        nc.tensor.transpose(pv[:],
                                            cnb[:, s * 128 : (s + 1) * 128],
                                            ident[:])
                        nc.vector.tensor_copy(cT[:, s, :], pv[:])
                    pc = pcp.tile([128, MQ, 128], F32, tag="pc", name="pc")
                    for m in range(MQ):
                        for t in range(KD // 2):
                            nc.tensor.matmul(pc[:, m, :],
                                             qT[:, 2 * t : 2 * t + 2,
                                                m * 128 : (m + 1) * 128],
                                             cT[:, 2 * t : 2 * t + 2, :],
                                             start=(t == 0), stop=False,
                                             perf_mode=DR)
                        nc.tensor.matmul(pc[:, m, :],
                                         qT6[:, m * 128 : (m + 1) * 128],
                                         cT6[:], start=False, stop=True)

            # ---------- finals ----------
            nc.vector.tensor_reduce(out=anmin[:], in_=anc[:], axis=AX.X, op=ALU.min)
            nc.vector.tensor_reduce(out=wmin[:], in_=pc[:], axis=AX.X, op=ALU.min)
            nc.vector.tensor_tensor(out=apw[:], in0=apw0[:], in1=apw1[:],
                                    op=ALU.max)
            ap2 = const.tile([128, MQ], F32)
            nc.vector.tensor_scalar_add(ap2[:], apw[:], -BIG)
            nc.vector.tensor_scalar_max(ap2[:], ap2[:], 1e-12)
            ap_d = const.tile([128, MQ], F32)
            nc.scalar.activation(out=ap_d[:], in_=ap2[:], func=ACTF.Sqrt)

            an2 = const.tile([128, MQ], F32)
            nc.vector.tensor_scalar_max(an2[:], anmin[:], 1e-12)
            an_d = const.tile([128, MQ], F32)
            nc.scalar.activation(out=an_d[:], in_=an2[:], func=ACTF.Sqrt)

            dc2 = const.tile([128, MQ], F32)
            nc.vector.tensor_scalar_max(dc2[:], wmin[:], 0.0)
            dc_d = const.tile([128, MQ], F32)
            nc.scalar.activation(out=dc_d[:], in_=dc2[:], func=ACTF.Sqrt)
            nc.vector.tensor_scalar_max(dc_d[:], dc_d[:], 1e-12)

            an_f = const.tile([128, MQ], F32)
            nc.vector.tensor_tensor(out=an_f[:], in0=an_d[:], in1=dc_d[:],
                                    op=ALU.min)
            diff = const.tile([128, MQ], F32)
            nc.vector.tensor_tensor(out=diff[:], in0=ap_d[:], in1=an_f[:],
                                    op=ALU.subtract)
            lvec = const.tile([128, MQ], F32)
            nc.scalar.activation(out=lvec[:], in_=diff[:], func=ACTF.Relu,
                                 bias=marg[:])
            lcol = const.tile([128, 1], F32)
            nc.vector.tensor_reduce(out=lcol[:], in_=lvec[:], axis=AX.X, op=ALU.add)
            lsum = const.tile([128, 1], F32)
            nc.gpsimd.partition_all_reduce(lsum[:], lcol[:], 128,
                                           bass_isa.ReduceOp.add)
            nc.sync.dma_start(out=loss_h[:], in_=lsum[0:1, 0:1])
            if dbg_h is not None:
                dbgt = const.tile([128, 8 * MQ], F32)
                nc.vector.tensor_copy(dbgt[:, 0:MQ], apw0[:])
                nc.vector.tensor_copy(dbgt[:, MQ:2 * MQ], apw1[:])
                nc.vector.tensor_copy(dbgt[:, 2 * MQ:3 * MQ], anmin[:])
                nc.vector.tensor_copy(dbgt[:, 3 * MQ:4 * MQ], wmin[:])
                nc.vector.tensor_copy(dbgt[:, 4 * MQ:5 * MQ], ap_d[:])
                nc.vector.tensor_copy(dbgt[:, 5 * MQ:6 * MQ], an_f[:])
                nc.vector.tensor_copy(dbgt[:, 6 * MQ:7 * MQ], lvec[:])
                nc.vector.tensor_copy(dbgt[:, 7 * MQ:8 * MQ], wb[:, 0:MQ])
                nc.sync.dma_start(out=dbg_h[:, :], in_=dbgt[:])

    nc.finalize()
    return nc


def _get_nc():
    global _nc_cache
    if _nc_cache is None:
        _nc_cache = _build()
    return _nc_cache


def _to_xT(xneg_rolled):
    # [N, D] key values -> [128, q*(KD*QW) + s*QW + j] quarter-blocked -X^T
    return np.ascontiguousarray(
        xneg_rolled.T.reshape(KD, 128, NQR, QW).transpose(1, 2, 0, 3)
        .reshape(128, KD * N)
    )


def _in_maps(inputs, targets, center):
    x = np.asarray(inputs, dtype=np.float32)
    t = np.asarray(targets).astype(np.int64).reshape(-1)
    c = np.ascontiguousarray(np.asarray(center, dtype=np.float32))
    assert x.shape == (N, D) and t.shape == (N,) and c.shape == (P, D)

    # sort rows by class (the loss is a mean over rows -> permutation
    # invariant); each query's same-class keys become one contiguous range
    perm = np.argsort(t, kind="stable")
    xs = x[perm]
    ts_ = t[perm]
    starts = np.searchsorted(ts_, ts_, side="left")
    ends = np.searchsorted(ts_, ts_, side="right")
    assert int((ends - starts).max()) <= 512, "class too large for max window"

    xneg = (-xs).astype(NPF8)                          # key values, fp8
    x2 = (2.0 * xs).astype(NPF8)                       # query values, fp8

    # aug row map: row 0 = sq_i, row 96 = sq_j/csq,
    # classes 0..94 -> rows 1..95, classes 95..99 -> rows 97..101
    rows = np.where(ts_ < 95, ts_ + 1, ts_ + 2)
    augk = np.zeros((128, N), dtype=NPBF16)
    augk[rows, np.arange(N)] = NPBF16(BIG)
    augk[0, :] = NPBF16(1.0)                           # sq_i coefficient

    # center aug rows: csq (=1) at row 96, huge for pad centers, sq_i coeff
    augc = np.zeros((128, 128), dtype=NPBF16)
    augc[96, 0:P] = NPBF16(1.0)
    augc[96, P:128] = NPBF16(1.0e6)
    augc[0, :] = NPBF16(1.0)

    maps = []
    for core in range(N_CORES):
        s = slice(core * NQ, (core + 1) * NQ)
        # rotate keys so this core's class neighborhood sits at cols [0,1536)
        shift = (core * NQ - 512) % N
        xT = _to_xT(np.roll(xneg, -shift, axis=0))
        augk_c = np.ascontiguousarray(np.roll(augk, -shift, axis=1))
        xq2T = np.ascontiguousarray(
            x2[s].T.reshape(KD, 128, NQ).transpose(1, 0, 2).reshape(128, KD * NQ)
        )
        augq = np.zeros((128, NQ), dtype=NPBF16)
        augq[rows[s], np.arange(NQ)] = NPBF16(1.0)
        augq[96, :] = NPBF16(1.0)                      # sq_j coefficient

        # window bounds per query row, in rotated coords, per chunk
        rl = (starts[s] - shift) % N
        rh = rl + (ends[s] - starts[s])
        assert rl.min() >= 0 and rh.max() <= 1536, "window escape"
        wb = np.zeros((128, MQ * 4), dtype=np.float32)
        for m in range(MQ):
            seg = slice(m * 128, (m + 1) * 128)
            wb[:, 4 * m + 0] = np.clip(rl[seg], 0, 1024)
            wb[:, 4 * m + 1] = np.clip(rh[seg], 0, 1024)
            wb[:, 4 * m + 2] = np.clip(rl[seg] - 1024, 0, 512)
            wb[:, 4 * m + 3] = np.clip(rh[seg] - 1024, 0, 512)

        maps.append({
            "xT": xT,
            "xq2T": xq2T,
            "augk": augk_c,
            "augq": augq,
            "augc": augc,
            "center": c,
            "wb": wb,
        })
    return maps


def run(inputs, targets, center, trace=False):
    nc = _get_nc()
    res = run_bass_kernel_spmd(
        nc, _in_maps(inputs, targets, center), list(range(N_CORES)), trace=trace
    )
    tot = sum(float(r["loss"][0, 0]) for r in res.results)
    loss = np.float32(tot / N)
    return np.asarray(loss), res


def kernel(inputs, targets, center):
    out, _ = run(inputs, targets, center)
    return out


# revision 58
# speedup vs baseline: 1.2538x; 1.0305x over previous
"""AugmentedTripletLoss Trainium2 kernel — 8-core SPMD, row-sharded.

Math (matches reference):
  d2[i,j] = sq_i + sq_j - 2*X@X.T
  ap_i    = sqrt(clip(max_{same class} d2, 1e-12))
  an_i    = min( sqrt(clip(min_{diff class} d2, 1e-12)),
                 clip(sqrt(clip(sq_i + csq_c - 2*x_i.cn_c, 0)), 1e-12) )
  loss    = mean(relu(1 + ap - an))

Strategy (per core, 512 query rows):
  Host marshals layouts only (transposes / sign scales / one-hot encodes —
  no FLOPs): keys as -X^T fp8 tiles (quarter-blocked for 12KB DMA rows),
  queries as 2*X_q^T fp8, one-hot class aug tiles in bf16.
  Main GEMM runs fp8 DoubleRow (2 contraction subtiles per matmul).
  Row norms are computed on-device from the same fp8 tiles: Scalar
  squares them into bf16, a ones-vector matmul row-reduces into a
  [1,512] PSUM row, and Scalar copies that into the bf16 aug rows
  (sq_j at aug row 96 against query coeff 1; sq_i at aug row 102
  against key coeff 1; BIG*onehot rows complete the aug tile).
  Each [128,1024] PSUM tile then holds u = d2 + BIG*[same class], so
  the masked max/min are plain DVE tensor_reduce passes. Work is
  quarter-pipelined over key columns. Centers: on-device normalize,
  negate, fp8 PE-transpose; csq/sq_i ride the aug rows.
  Final: per-core partial sum -> host gathers the 8 scalars, sums, /N.
"""
import os
import sys

for _p in ("/opt/trn_rl_repo", "/root/.axon_site"):
    if _p not in sys.path:
        sys.path.insert(0, _p)

import numpy as np
import ml_dtypes

import concourse.bass as bass
import concourse.bacc as bacc
import concourse.mybir as mybir
import concourse.bass_isa as bass_isa
from concourse.tile import TileContext
from concourse.masks import make_identity
from concourse.bass_utils import run_bass_kernel_spmd
from concourse.dve_ops import TENSOR_MASK_REDUCE as TMR

F32 = mybir.dt.float32
BF16 = mybir.dt.bfloat16
F8 = mybir.dt.float8e4
ALU = mybir.AluOpType
ACTF = mybir.ActivationFunctionType
AX = mybir.AxisListType
DR = mybir.MatmulPerfMode.DoubleRow
NPBF16 = ml_dtypes.bfloat16
NPF8 = ml_dtypes.float8_e4m3

N_CORES = 8
N, D, P = 4096, 768, 100
NQ = N // N_CORES        # 512 query rows per core
MQ = NQ // 128           # 4 query m-tiles
KD = D // 128            # 6 contraction tiles
NQR = 4                  # key-column quarters
QW = N // NQR            # 1024 cols per quarter
BIG = 16384.0
MARGIN = 1.0

_nc_cache = None


def _build():
    nc = bacc.Bacc("TRN2", target_bir_lowering=False, num_devices=N_CORES)

    # xT: quarter-blocked -X^T fp8; col q*(KD*QW) + s*QW + j = -x[q*QW+j, 128s+p]
    xT_h = nc.declare_dram_parameter("xT", [128, KD * N], F8, isOutput=False)
    xq2T_h = nc.declare_dram_parameter("xq2T", [128, KD * NQ], F8, isOutput=False)
    augk_h = nc.declare_dram_parameter("augk", [128, N], BF16, isOutput=False)
    augq_h = nc.declare_dram_parameter("augq", [128, NQ], BF16, isOutput=False)
    augc_h = nc.declare_dram_parameter("augc", [128, 128], BF16, isOutput=False)
    # per-row same-class window bounds in rotated key coords:
    # [p, 4m+{0,1,2,3}] = lo/hi within cols 0:1024, lo/hi within cols 1024:1536
    wb_h = nc.declare_dram_parameter("wb", [128, MQ * 4], F32, isOutput=False)
    cen_h = nc.declare_dram_parameter("center", [P, D], F32, isOutput=False)
    loss_h = nc.declare_dram_parameter("loss", [1, 1], F32, isOutput=True)
    dbg_h = (nc.declare_dram_parameter("dbg", [128, 8 * MQ], F32, isOutput=True)
             if os.environ.get("KDBG") == "1" else None)

    with TileContext(nc) as tc:
        from contextlib import ExitStack

        with ExitStack() as ctx:
            const = ctx.enter_context(tc.tile_pool(name="const", bufs=1))
            ksqp = ctx.enter_context(tc.tile_pool(name="ksqp", bufs=2))
            pmain = ctx.enter_context(tc.tile_pool(name="pmain", bufs=3, space="PSUM"))
            psmall = ctx.enter_context(tc.tile_pool(name="psmall", bufs=1,
                                                    space="PSUM"))
            pcp = ctx.enter_context(tc.tile_pool(name="pcp", bufs=1,
                                                 space="PSUM"))

            # ---------- persistent tiles ----------
            kT = [const.tile([128, KD, QW], F8, name=f"kT{q}")
                  for q in range(NQR)]                 # -X^T keys, per quarter
            kT6 = const.tile([128, N], BF16)           # aug keys
            qT = const.tile([128, KD, NQ], F8)         # 2*X_q^T query tiles
            qT6 = const.tile([128, NQ], BF16)          # aug queries
            qsq = const.tile([128, KD, NQ], BF16)      # squared query tiles
            onek = const.tile([128, 1], BF16)
            oneq = const.tile([128, 1], BF16)
            ident = const.tile([128, 128], BF16)
            ct32 = const.tile([128, D], F32)
            cdump = const.tile([128, D], F32)
            csum = const.tile([128, 1], F32)
            cnorm = const.tile([128, 1], F32)
            rnorm = const.tile([128, 1], F32)
            cn32 = const.tile([128, D], F32)
            cnb = const.tile([128, D], BF16)
            cT = const.tile([128, KD, 128], F8)        # -cn^T tiles
            cT6 = const.tile([128, 128], BF16)         # center aug rows
            eps30 = const.tile([128, 1], F32)
            marg = const.tile([128, 1], F32)
            anc = const.tile([128, MQ, NQR], F32)
            anmin = const.tile([128, MQ], F32)
            wmin = const.tile([128, MQ], F32)
            wb = const.tile([128, MQ * 4], F32)
            apw0 = const.tile([128, MQ], F32)
            apw1 = const.tile([128, MQ], F32)
            apw = const.tile([128, MQ], F32)
            mscr = const.tile([128, QW], F32)           # mask-reduce out dump

            # ---------- sync engine: all load DMAs ----------
            for t in range(KD // 2):
                nc.sync.dma_start(
                    out=qT[:, 2 * t : 2 * t + 2, :].rearrange("p s n -> p (s n)"),
                    in_=xq2T_h[:, 2 * t * NQ : (2 * t + 2) * NQ])
            for q in range(NQR):
                for t in range(KD // 2):
                    nc.sync.dma_start(
                        out=kT[q][:, 2 * t : 2 * t + 2, :].rearrange(
                            "p s n -> p (s n)"),
                        in_=xT_h[:, q * KD * QW + 2 * t * QW :
                                 q * KD * QW + (2 * t + 2) * QW],
                    )
                if q == 0:
                    nc.sync.dma_start(out=kT6[:], in_=augk_h[:, :])
                    nc.sync.dma_start(out=qT6[:], in_=augq_h[:, :])
                    nc.sync.dma_start(out=cT6[:], in_=augc_h[:, :])
                    nc.sync.dma_start(out=ct32[0:P, :], in_=cen_h[:, :])
                    nc.sync.dma_start(out=wb[:], in_=wb_h[:, :])

            # ---------- vector: init ----------
            nc.vector.memset(ct32[96:128, :], 0.0)
            nc.vector.memset(eps30[:], 1e-30)
            nc.vector.memset(marg[:], MARGIN)
            nc.vector.memset(onek[:], 1.0)
            nc.vector.memset(oneq[:], 0.25)            # undo the 2x query scale
            make_identity(nc, ident[:])

            # ---------- row-norm chain: squares -> ones-matmul -> aug rows --
            # query side first (gates center GEMM and all aug matmuls)
            for t in range(KD // 2):
                nc.scalar.activation(
                    out=qsq[:, 2 * t : 2 * t + 2, :].rearrange("p s n -> p (s n)"),
                    in_=qT[:, 2 * t : 2 * t + 2, :].rearrange("p s n -> p (s n)"),
                    func=ACTF.Square)
            qrow = psmall.tile([1, NQ], F32, tag="sq", name="qrow")
            for s in range(KD):
                nc.tensor.matmul(qrow[:], oneq[:], qsq[:, s, :],
                                 start=(s == 0), stop=(s == KD - 1))
            nc.vector.tensor_copy(qT6[0:1, :], qrow[:])

            # ---------- main GEMM: quarters x m-tiles (fp8 DoubleRow) -------
            for q in range(NQR):
                # squares for this quarter, split by jj half for latency
                ksq = ksqp.tile([128, KD, QW], BF16, tag="ksq", name=f"ksq{q}")
                for jj in range(QW // 512):
                    for t in range(KD // 2):
                        nc.scalar.activation(
                            out=ksq[:, 2 * t : 2 * t + 2,
                                    jj * 512 : (jj + 1) * 512],
                            in_=kT[q][:, 2 * t : 2 * t + 2,
                                      jj * 512 : (jj + 1) * 512],
                            func=ACTF.Square)

                # d<6 for m0..m2, then their augs, then m3 (3 PSUM bufs)
                def emit_main(m, pt):
                    for t in range(KD // 2):
                        lhsT = qT[:, 2 * t : 2 * t + 2, m * 128 : (m + 1) * 128]
                        for jj in range(QW // 512):
                            nc.tensor.matmul(
                                pt[:, jj * 512 : (jj + 1) * 512], lhsT,
                                kT[q][:, 2 * t : 2 * t + 2,
                                      jj * 512 : (jj + 1) * 512],
                                start=(t == 0), stop=False, perf_mode=DR,
                            )

                def emit_aug(m, pt):
                    lhsT = qT6[:, m * 128 : (m + 1) * 128]
                    for jj in range(QW // 512):
                        nc.tensor.matmul(
                            pt[:, jj * 512 : (jj + 1) * 512], lhsT,
                            kT6[:, q * QW + jj * 512 : q * QW + (jj + 1) * 512],
                            start=False, stop=True,
                        )

                def emit_red(m, pt):
                    nc.vector.tensor_reduce(out=anc[:, m, q : q + 1],
                                            in_=pt[:], axis=AX.X, op=ALU.min)
                    # same-class max: rotated layout puts every query's class
                    # window inside key cols [0, 1536) = q0 + first half of q1.
                    # Table-based custom DVE op; seed must be an immediate
                    # (a per-partition s1 AP wedges the exec unit), so the two
                    # chunks accumulate separately and finals max them.
                    if q == 0:
                        nc.vector._custom_dve(
                            TMR, out=mscr[:], in0=pt[:],
                            in1=wb[:, 4 * m + 1 : 4 * m + 2],
                            s0=wb[:, 4 * m : 4 * m + 1],
                            s1=-3.0e38, imm2=1.0,
                            accum_out=apw0[:, m : m + 1],
                        )
                    elif q == 1:
                        nc.vector._custom_dve(
                            TMR, out=mscr[:, 0:512], in0=pt[:, 0:512],
                            in1=wb[:, 4 * m + 3 : 4 * m + 4],
                            s0=wb[:, 4 * m + 2 : 4 * m + 3],
                            s1=-3.0e38, imm2=1.0,
                            accum_out=apw1[:, m : m + 1],
                        )

                pts = []
                for m in range(MQ - 1):
                    pt = pmain.tile([128, QW], F32, tag="mm", name=f"pt{q}_{m}")
                    emit_main(m, pt)
                    pts.append(pt)
                # key row-norm matmuls after d<6 (ksq lands during them)
                for jj in range(QW // 512):
                    krow = psmall.tile([1, 512], F32, tag="sq",
                                       name=f"krow{q}_{jj}")
                    for s in range(KD):
                        nc.tensor.matmul(
                            krow[:], onek[:],
                            ksq[:, s, jj * 512 : (jj + 1) * 512],
                            start=(s == 0), stop=(s == KD - 1))
                    dst = kT6[96:97, q * QW + jj * 512 : q * QW + (jj + 1) * 512]
                    if q == 0:
                        nc.vector.tensor_copy(dst, krow[:])
                    else:
                        nc.scalar.activation(out=dst, in_=krow[:],
                                             func=ACTF.Copy)
                for m in range(MQ - 1):
                    emit_aug(m, pts[m])
                for m in range(MQ - 1):
                    emit_red(m, pts[m])
                m = MQ - 1
                pt = pmain.tile([128, QW], F32, tag="mm", name=f"pt{q}_{m}")
                emit_main(m, pt)
                emit_aug(m, pt)
                emit_red(m, pt)

                if q == 0:
                    # center normalize chain + GEMM slot into the post-q0
                    # bubble (scalar/vector/PE all have slack here)
                    nc.scalar.activation(out=cdump[:], in_=ct32[:],
                                         func=ACTF.Square, accum_out=csum[:])
                    nc.scalar.activation(out=cnorm[:], in_=csum[:],
                                         func=ACTF.Sqrt, bias=eps30[:])
                    nc.vector.reciprocal(rnorm[:], cnorm[:])
                    nc.vector.tensor_scalar_mul(rnorm[:], rnorm[:], -1.0)
                    nc.vector.tensor_scalar(out=cn32[:], in0=ct32[:],
                                            scalar1=rnorm[:, 0:1], scalar2=None,
                                            op0=ALU.mult)
                    nc.vector.tensor_copy(cnb[:], cn32[:])
                    for s in range(KD):
                        pv = pcp.tile([128, 128], BF16, tag="pc",
                                      name=f"ctr{s}")
                        nc.tensor.transpose(pv[:],
                                            cnb[:, s * 128 : (s + 1) * 128],
                                            ident[:])
                        nc.vector.tensor_copy(cT[:, s, :], pv[:])
                    pc = pcp.tile([128, MQ, 128], F32, tag="pc", name="pc")
                    for m in range(MQ):
                        for t in range(KD // 2):
                            nc.tensor.matmul(pc[:, m, :],
                                             qT[:, 2 * t : 2 * t + 2,
                                                m * 128 : (m + 1) * 128],
                                             cT[:, 2 * t : 2 * t + 2, :],
                                             start=(t == 0), stop=False,
                                             perf_mode=DR)
                        nc.tensor.matmul(pc[:, m, :],
                                         qT6[:, m * 128 : (m + 1) * 128],
                                         cT6[:], start=False, stop=True)

            # ---------- finals ----------
            nc.vector.tensor_reduce(out=anmin[:], in_=anc[:], axis=AX.X, op=ALU.min)
            nc.vector.tensor_reduce(out=wmin[:], in_=pc[:], axis=AX.X, op=ALU.min)
            nc.vector.tensor_tensor(out=apw[:], in0=apw0[:], in1=apw1[:],
                                    op=ALU.max)
            ap2 = const.tile([128, MQ], F32)
            nc.vector.tensor_scalar_add(ap2[:], apw[:], -BIG)
            nc.vector.tensor_scalar_max(ap2[:], ap2[:], 1e-12)
            ap_d = const.tile([128, MQ], F32)
            nc.scalar.activation(out=ap_d[:], in_=ap2[:], func=ACTF.Sqrt)

            an2 = const.tile([128, MQ], F32)
            nc.vector.tensor_scalar_max(an2[:], anmin[:], 1e-12)
            an_d = const.tile([128, MQ], F32)
            nc.scalar.activation(out=an_d[:], in_=an2[:], func=ACTF.Sqrt)

            dc2 = const.tile([128, MQ], F32)
            nc.vector.tensor_scalar_max(dc2[:], wmin[:], 0.0)
            dc_d = const.tile([128, MQ], F32)
            nc.scalar.activation(out=dc_d[:], in_=dc2[:], func=ACTF.Sqrt)
            nc.vector.tensor_scalar_max(dc_d[:], dc_d[:], 1e-12)

            an_f = const.tile([128, MQ], F32)
            nc.vector.tensor_tensor(out=an_f[:], in0=an_d[:], in1=dc_d[:],
                                    op=ALU.min)
            diff = const.tile([128, MQ], F32)
            nc.vector.tensor_tensor(out=diff[:], in0=ap_d[:], in1=an_f[:],
                                    op=ALU.subtract)
            lvec = const.tile([128, MQ], F32)
            nc.scalar.activation(out=lvec[:], in_=diff[:], func=ACTF.Relu,
                                 bias=marg[:])
            lcol = const.tile([128, 1], F32)
            nc.vector.tensor_reduce(out=lcol[:], in_=lvec[:], axis=AX.X, op=ALU.add)
            lsum = const.tile([128, 1], F32)
            nc.gpsimd.partition_all_reduce(lsum[:], lcol[:], 128,
                                           bass_isa.ReduceOp.add)
            nc.sync.dma_start(out=loss_h[:], in_=lsum[0:1, 0:1])
            if dbg_h is not None:
                dbgt = const.tile([128, 8 * MQ], F32)
                nc.vector.tensor_copy(dbgt[:, 0:MQ], apw0[:])
                nc.vector.tensor_copy(dbgt[:, MQ:2 * MQ], apw1[:])
                nc.vector.tensor_copy(dbgt[:, 2 * MQ:3 * MQ], anmin[:])
                nc.vector.tensor_copy(dbgt[:, 3 * MQ:4 * MQ], wmin[:])
                nc.vector.tensor_copy(dbgt[:, 4 * MQ:5 * MQ], ap_d[:])
                nc.vector.tensor_copy(dbgt[:, 5 * MQ:6 * MQ], an_f[:])
                nc.vector.tensor_copy(dbgt[:, 6 * MQ:7 * MQ], lvec[:])
                nc.vector.tensor_copy(dbgt[:, 7 * MQ:8 * MQ], wb[:, 0:MQ])
                nc.sync.dma_start(out=dbg_h[:, :], in_=dbgt[:])

    nc.finalize()
    return nc


def _get_nc():
    global _nc_cache
    if _nc_cache is None:
        _nc_cache = _build()
    return _nc_cache


def _to_xT(xneg_rolled):
    # [N, D] key values -> [128, q*(KD*QW) + s*QW + j] quarter-blocked -X^T
    return np.ascontiguousarray(
        xneg_rolled.T.reshape(KD, 128, NQR, QW).transpose(1, 2, 0, 3)
        .reshape(128, KD * N)
    )


def _in_maps(inputs, targets, center):
    x = np.asarray(inputs, dtype=np.float32)
    t = np.asarray(targets).astype(np.int64).reshape(-1)
    c = np.ascontiguousarray(np.asarray(center, dtype=np.float32))
    assert x.shape == (N, D) and t.shape == (N,) and c.shape == (P, D)

    # sort rows by class (the loss is a mean over rows -> permutation
    # invariant); each query's same-class keys become one contiguous range
    perm = np.argsort(t, kind="stable")
    xs = x[perm]
    ts_ = t[perm]
    starts = np.searchsorted(ts_, ts_, side="left")
    ends = np.searchsorted(ts_, ts_, side="right")
    assert int((ends - starts).max()) <= 512, "class too large for max window"

    xneg = (-xs).astype(NPF8)                          # key values, fp8
    x2 = (2.0 * xs).astype(NPF8)                       # query values, fp8

    # aug row map: row 0 = sq_i, row 96 = sq_j/csq,
    # classes 0..94 -> rows 1..95, classes 95..99 -> rows 97..101
    rows = np.where(ts_ < 95, ts_ + 1, ts_ + 2)
    augk = np.zeros((128, N), dtype=NPBF16)
    augk[rows, np.arange(N)] = NPBF16(BIG)
    augk[0, :] = NPBF16(1.0)                           # sq_i coefficient

    # center aug rows: csq (=1) at row 96, huge for pad centers, sq_i coeff
    augc = np.zeros((128, 128), dtype=NPBF16)
    augc[96, 0:P] = NPBF16(1.0)
    augc[96, P:128] = NPBF16(1.0e6)
    augc[0, :] = NPBF16(1.0)

    maps = []
    for core in range(N_CORES):
        s = slice(core * NQ, (core + 1) * NQ)
        # rotate keys so this core's class neighborhood sits at cols [0,1536)
        shift = (core * NQ - 512) % N
        xT = _to_xT(np.roll(xneg, -shift, axis=0))
        augk_c = np.ascontiguousarray(np.roll(augk, -shift, axis=1))
        xq2T = np.ascontiguousarray(
            x2[s].T.reshape(KD, 128, NQ).transpose(1, 0, 2).reshape(128, KD * NQ)
        )
        augq = np.zeros((128, NQ), dtype=NPBF16)
        augq[rows[s], np.arange(NQ)] = NPBF16(1.0)
        augq[96, :] = NPBF16(1.0)                      # sq_j coefficient

        # window bounds per query row, in rotated coords, per chunk
        rl = (starts[s] - shift) % N
        rh = rl + (ends[s] - starts[s])
        assert rl.min() >= 0 and rh.max() <= 1536, "window escape"
        wb = np.zeros((128, MQ * 4), dtype=np.float32)
        for m in range(MQ):
            seg = slice(m * 128, (m + 1) * 128)
            wb[:, 4 * m + 0] = np.clip(rl[seg], 0, 1024)
            wb[:, 4 * m + 1] = np.clip(rh[seg], 0, 1024)
            wb[:, 4 * m + 2] = np.clip(rl[seg] - 1024, 0, 512)
            wb[:, 4 * m + 3] = np.clip(rh[seg] - 1024, 0, 512)

        maps.append({
            "xT": xT,
            "xq2T": xq2T,
            "augk": augk_c,
            "augq": augq,
            "augc": augc,
            "center": c,
            "wb": wb,
        })
    return maps


def run(inputs, targets, center, trace=False):
    nc = _get_nc()
    res = run_bass_kernel_spmd(
        nc, _in_maps(inputs, targets, center), list(range(N_CORES)), trace=trace
    )
    tot = sum(float(r["loss"][0, 0]) for r in res.results)
    loss = np.float32(tot / N)
    return np.asarray(loss), res


def kernel(inputs, targets, center):
    out, _ = run(inputs, targets, center)
    return out


# revision 59
# speedup vs baseline: 1.3101x; 1.0448x over previous
"""AugmentedTripletLoss Trainium2 kernel — 8-core SPMD, row-sharded.

Math (matches reference):
  d2[i,j] = sq_i + sq_j - 2*X@X.T
  ap_i    = sqrt(clip(max_{same class} d2, 1e-12))
  an_i    = min( sqrt(clip(min_{diff class} d2, 1e-12)),
                 clip(sqrt(clip(sq_i + csq_c - 2*x_i.cn_c, 0)), 1e-12) )
  loss    = mean(relu(1 + ap - an))

Strategy (per core, 512 query rows):
  Host marshals layouts only (transposes / sign scales / one-hot encodes —
  no FLOPs): keys as -X^T fp8 tiles (quarter-blocked for 12KB DMA rows),
  queries as 2*X_q^T fp8, one-hot class aug tiles in bf16.
  Main GEMM runs fp8 DoubleRow (2 contraction subtiles per matmul).
  Row norms are computed on-device from the same fp8 tiles: Scalar
  squares them into bf16, a ones-vector matmul row-reduces into a
  [1,512] PSUM row, and Scalar copies that into the bf16 aug rows
  (sq_j at aug row 96 against query coeff 1; sq_i at aug row 102
  against key coeff 1; BIG*onehot rows complete the aug tile).
  Each [128,1024] PSUM tile then holds u = d2 + BIG*[same class], so
  the masked max/min are plain DVE tensor_reduce passes. Work is
  quarter-pipelined over key columns. Centers: on-device normalize,
  negate, fp8 PE-transpose; csq/sq_i ride the aug rows.
  Final: per-core partial sum -> host gathers the 8 scalars, sums, /N.
"""
import os
import sys

for _p in ("/opt/trn_rl_repo", "/root/.axon_site"):
    if _p not in sys.path:
        sys.path.insert(0, _p)

import numpy as np
import ml_dtypes

import concourse.bass as bass
import concourse.bacc as bacc
import concourse.mybir as mybir
import concourse.bass_isa as bass_isa
from concourse.tile import TileContext
from concourse.masks import make_identity
from concourse.bass_utils import run_bass_kernel_spmd
from concourse.dve_ops import TENSOR_MASK_REDUCE as TMR

F32 = mybir.dt.float32
BF16 = mybir.dt.bfloat16
F8 = mybir.dt.float8e4
ALU = mybir.AluOpType
ACTF = mybir.ActivationFunctionType
AX = mybir.AxisListType
DR = mybir.MatmulPerfMode.DoubleRow
NPBF16 = ml_dtypes.bfloat16
NPF8 = ml_dtypes.float8_e4m3

N_CORES = 8
N, D, P = 4096, 768, 100
NQ = N // N_CORES        # 512 query rows per core
MQ = NQ // 128           # 4 query m-tiles
KD = D // 128            # 6 contraction tiles
NQR = 4                  # key-column quarters
QW = N // NQR            # 1024 cols per quarter
BIG = 16384.0
MARGIN = 1.0

_nc_cache = None


def _build():
    nc = bacc.Bacc("TRN2", target_bir_lowering=False, num_devices=N_CORES)

    # xT: quarter-blocked -X^T fp8; col q*(KD*QW) + s*QW + j = -x[q*QW+j, 128s+p]
    xT_h = nc.declare_dram_parameter("xT", [128, KD * N], F8, isOutput=False)
    xq2T_h = nc.declare_dram_parameter("xq2T", [128, KD * NQ], F8, isOutput=False)
    augk_h = nc.declare_dram_parameter("augk", [128, N], BF16, isOutput=False)
    augq_h = nc.declare_dram_parameter("augq", [128, NQ], BF16, isOutput=False)
    augc_h = nc.declare_dram_parameter("augc", [128, 128], BF16, isOutput=False)
    # per-row same-class window bounds in rotated key coords:
    # [p, 4m+{0,1,2,3}] = lo/hi within cols 0:1024, lo/hi within cols 1024:1536
    wb_h = nc.declare_dram_parameter("wb", [128, MQ * 4], F32, isOutput=False)
    cen_h = nc.declare_dram_parameter("center", [P, D], F32, isOutput=False)
    loss_h = nc.declare_dram_parameter("loss", [1, 1], F32, isOutput=True)
    dbg_h = (nc.declare_dram_parameter("dbg", [128, 8 * MQ], F32, isOutput=True)
             if os.environ.get("KDBG") == "1" else None)

    with TileContext(nc) as tc:
        from contextlib import ExitStack

        with ExitStack() as ctx:
            const = ctx.enter_context(tc.tile_pool(name="const", bufs=1))
            ksqp = ctx.enter_context(tc.tile_pool(name="ksqp", bufs=2))
            pmain = ctx.enter_context(tc.tile_pool(name="pmain", bufs=3, space="PSUM"))
            psmall = ctx.enter_context(tc.tile_pool(name="psmall", bufs=1,
                                                    space="PSUM"))
            pcp = ctx.enter_context(tc.tile_pool(name="pcp", bufs=1,
                                                 space="PSUM"))

            # ---------- persistent tiles ----------
            kT = [const.tile([128, KD, QW], F8, name=f"kT{q}")
                  for q in range(NQR)]                 # -X^T keys, per quarter
            kT6 = const.tile([128, N], BF16)           # aug keys
            qT = const.tile([128, KD, NQ], F8)         # 2*X_q^T query tiles
            qT6 = const.tile([128, NQ], BF16)          # aug queries
            qsq = const.tile([128, KD, NQ], F8)        # squared query tiles
            onek = const.tile([128, 2, 128], F8)
            oneq = const.tile([128, 2, 128], F8)
            ident = const.tile([128, 128], BF16)
            ct32 = const.tile([128, D], F32)
            cdump = const.tile([128, D], F32)
            csum = const.tile([128, 1], F32)
            cnorm = const.tile([128, 1], F32)
            rnorm = const.tile([128, 1], F32)
            cn32 = const.tile([128, D], F32)
            cnb = const.tile([128, D], BF16)
            cT = const.tile([128, KD, 128], F8)        # -cn^T tiles
            cT6 = const.tile([128, 128], BF16)         # center aug rows
            eps30 = const.tile([128, 1], F32)
            marg = const.tile([128, 1], F32)
            anc = const.tile([128, MQ, NQR], F32)
            anmin = const.tile([128, MQ], F32)
            wmin = const.tile([128, MQ], F32)
            wb = const.tile([128, MQ * 4], F32)
            apw0 = const.tile([128, MQ], F32)
            apw1 = const.tile([128, MQ], F32)
            apw = const.tile([128, MQ], F32)
            mscr = const.tile([128, QW], F32)           # mask-reduce out dump

            # ---------- sync engine: all load DMAs ----------
            for t in range(KD // 2):
                nc.sync.dma_start(
                    out=qT[:, 2 * t : 2 * t + 2, :].rearrange("p s n -> p (s n)"),
                    in_=xq2T_h[:, 2 * t * NQ : (2 * t + 2) * NQ])
            for q in range(NQR):
                for t in range(KD // 2):
                    nc.sync.dma_start(
                        out=kT[q][:, 2 * t : 2 * t + 2, :].rearrange(
                            "p s n -> p (s n)"),
                        in_=xT_h[:, q * KD * QW + 2 * t * QW :
                                 q * KD * QW + (2 * t + 2) * QW],
                    )
                if q == 0:
                    nc.sync.dma_start(out=kT6[:], in_=augk_h[:, :])
                    nc.sync.dma_start(out=qT6[:], in_=augq_h[:, :])
                    nc.sync.dma_start(out=cT6[:], in_=augc_h[:, :])
                    nc.sync.dma_start(out=ct32[0:P, :], in_=cen_h[:, :])
                    nc.sync.dma_start(out=wb[:], in_=wb_h[:, :])

            # ---------- vector: init ----------
            nc.vector.memset(ct32[96:128, :], 0.0)
            nc.vector.memset(eps30[:], 1e-30)
            nc.vector.memset(marg[:], MARGIN)
            nc.vector.memset(onek[:], 1.0)
            nc.vector.memset(oneq[:], 0.25)            # undo the 2x query scale
            make_identity(nc, ident[:])

            # ---------- row-norm chain: squares -> ones-matmul -> aug rows --
            # query side first (gates center GEMM and all aug matmuls)
            for t in range(KD // 2):
                nc.scalar.activation(
                    out=qsq[:, 2 * t : 2 * t + 2, :].rearrange("p s n -> p (s n)"),
                    in_=qT[:, 2 * t : 2 * t + 2, :].rearrange("p s n -> p (s n)"),
                    func=ACTF.Square)
            qrow = psmall.tile([128, NQ], F32, tag="sq", name="qrow")
            for t in range(KD // 2):
                nc.tensor.matmul(qrow[:], oneq[:], qsq[:, 2 * t : 2 * t + 2, :],
                                 start=(t == 0), stop=(t == KD // 2 - 1),
                                 perf_mode=DR)
            nc.vector.tensor_copy(qT6[0:1, :], qrow[0:1, :])

            # ---------- main GEMM: quarters x m-tiles (fp8 DoubleRow) -------
            for q in range(NQR):
                # squares for this quarter, split by jj half for latency
                ksq = ksqp.tile([128, KD, QW], F8, tag="ksq", name=f"ksq{q}")
                for jj in range(QW // 512):
                    for t in range(KD // 2):
                        nc.scalar.activation(
                            out=ksq[:, 2 * t : 2 * t + 2,
                                    jj * 512 : (jj + 1) * 512],
                            in_=kT[q][:, 2 * t : 2 * t + 2,
                                      jj * 512 : (jj + 1) * 512],
                            func=ACTF.Square)

                # d<6 for m0..m2, then their augs, then m3 (3 PSUM bufs)
                def emit_main(m, pt):
                    for t in range(KD // 2):
                        lhsT = qT[:, 2 * t : 2 * t + 2, m * 128 : (m + 1) * 128]
                        for jj in range(QW // 512):
                            nc.tensor.matmul(
                                pt[:, jj * 512 : (jj + 1) * 512], lhsT,
                                kT[q][:, 2 * t : 2 * t + 2,
                                      jj * 512 : (jj + 1) * 512],
                                start=(t == 0), stop=False, perf_mode=DR,
                            )

                def emit_aug(m, pt):
                    lhsT = qT6[:, m * 128 : (m + 1) * 128]
                    for jj in range(QW // 512):
                        nc.tensor.matmul(
                            pt[:, jj * 512 : (jj + 1) * 512], lhsT,
                            kT6[:, q * QW + jj * 512 : q * QW + (jj + 1) * 512],
                            start=False, stop=True,
                        )

                def emit_red(m, pt):
                    nc.vector.tensor_reduce(out=anc[:, m, q : q + 1],
                                            in_=pt[:], axis=AX.X, op=ALU.min)
                    # same-class max: rotated layout puts every query's class
                    # window inside key cols [0, 1536) = q0 + first half of q1.
                    # Table-based custom DVE op; seed must be an immediate
                    # (a per-partition s1 AP wedges the exec unit), so the two
                    # chunks accumulate separately and finals max them.
                    if q == 0:
                        nc.vector._custom_dve(
                            TMR, out=mscr[:], in0=pt[:],
                            in1=wb[:, 4 * m + 1 : 4 * m + 2],
                            s0=wb[:, 4 * m : 4 * m + 1],
                            s1=-3.0e38, imm2=1.0,
                            accum_out=apw0[:, m : m + 1],
                        )
                    elif q == 1:
                        nc.vector._custom_dve(
                            TMR, out=mscr[:, 0:512], in0=pt[:, 0:512],
                            in1=wb[:, 4 * m + 3 : 4 * m + 4],
                            s0=wb[:, 4 * m + 2 : 4 * m + 3],
                            s1=-3.0e38, imm2=1.0,
                            accum_out=apw1[:, m : m + 1],
                        )

                pts = []
                for m in range(MQ - 1):
                    pt = pmain.tile([128, QW], F32, tag="mm", name=f"pt{q}_{m}")
                    emit_main(m, pt)
                    pts.append(pt)
                # key row-norm matmuls after d<6 (ksq lands during them)
                for jj in range(QW // 512):
                    krow = psmall.tile([128, 512], F32, tag="sq",
                                       name=f"krow{q}_{jj}")
                    for t in range(KD // 2):
                        nc.tensor.matmul(
                            krow[:], onek[:],
                            ksq[:, 2 * t : 2 * t + 2,
                                jj * 512 : (jj + 1) * 512],
                            start=(t == 0), stop=(t == KD // 2 - 1),
                            perf_mode=DR)
                    dst = kT6[96:97, q * QW + jj * 512 : q * QW + (jj + 1) * 512]
                    if q == 0:
                        nc.vector.tensor_copy(dst, krow[0:1, :])
                    else:
                        nc.scalar.activation(out=dst, in_=krow[0:1, :],
                                             func=ACTF.Copy)
                for m in range(MQ - 1):
                    emit_aug(m, pts[m])
                for m in range(MQ - 1):
                    emit_red(m, pts[m])
                m = MQ - 1
                pt = pmain.tile([128, QW], F32, tag="mm", name=f"pt{q}_{m}")
                emit_main(m, pt)
                emit_aug(m, pt)
                emit_red(m, pt)

                if q == 0:
                    # center normalize chain + GEMM slot into the post-q0
                    # bubble (scalar/vector/PE all have slack here)
                    nc.scalar.activation(out=cdump[:], in_=ct32[:],
                                         func=ACTF.Square, accum_out=csum[:])
                    nc.scalar.activation(out=cnorm[:], in_=csum[:],
                                         func=ACTF.Sqrt, bias=eps30[:])
                    nc.vector.reciprocal(rnorm[:], cnorm[:])
                    nc.vector.tensor_scalar_mul(rnorm[:], rnorm[:], -1.0)
                    nc.vector.tensor_scalar(out=cn32[:], in0=ct32[:],
                                            scalar1=rnorm[:, 0:1], scalar2=None,
                                            op0=ALU.mult)
                    nc.vector.tensor_copy(cnb[:], cn32[:])
                    for s in range(KD):
                        pv = pcp.tile([128, 128], BF16, tag="pc",
                                      name=f"ctr{s}")
                        nc.tensor.transpose(pv[:],
                                            cnb[:, s * 128 : (s + 1) * 128],
                                            ident[:])
                        nc.vector.tensor_copy(cT[:, s, :], pv[:])
                    pc = pcp.tile([128, MQ, 128], F32, tag="pc", name="pc")
                    for m in range(MQ):
                        for t in range(KD // 2):
                            nc.tensor.matmul(pc[:, m, :],
                                             qT[:, 2 * t : 2 * t + 2,
                                                m * 128 : (m + 1) * 128],
                                             cT[:, 2 * t : 2 * t + 2, :],
                                             start=(t == 0), stop=False,
                                             perf_mode=DR)
                        nc.tensor.matmul(pc[:, m, :],
                                         qT6[:, m * 128 : (m + 1) * 128],
                                         cT6[:], start=False, stop=True)

            # ---------- finals ----------
            nc.vector.tensor_reduce(out=anmin[:], in_=anc[:], axis=AX.X, op=ALU.min)
            nc.vector.tensor_reduce(out=wmin[:], in_=pc[:], axis=AX.X, op=ALU.min)
            nc.vector.tensor_tensor(out=apw[:], in0=apw0[:], in1=apw1[:],
                                    op=ALU.max)
            ap2 = const.tile([128, MQ], F32)
            nc.vector.tensor_scalar_add(ap2[:], apw[:], -BIG)
            nc.vector.tensor_scalar_max(ap2[:], ap2[:], 1e-12)
            ap_d = const.tile([128, MQ], F32)
            nc.scalar.activation(out=ap_d[:], in_=ap2[:], func=ACTF.Sqrt)

            an2 = const.tile([128, MQ], F32)
            nc.vector.tensor_scalar_max(an2[:], anmin[:], 1e-12)
            an_d = const.tile([128, MQ], F32)
            nc.scalar.activation(out=an_d[:], in_=an2[:], func=ACTF.Sqrt)

            dc2 = const.tile([128, MQ], F32)
            nc.vector.tensor_scalar_max(dc2[:], wmin[:], 0.0)
            dc_d = const.tile([128, MQ], F32)
            nc.scalar.activation(out=dc_d[:], in_=dc2[:], func=ACTF.Sqrt)
            nc.vector.tensor_scalar_max(dc_d[:], dc_d[:], 1e-12)

            an_f = const.tile([128, MQ], F32)
            nc.vector.tensor_tensor(out=an_f[:], in0=an_d[:], in1=dc_d[:],
                                    op=ALU.min)
            diff = const.tile([128, MQ], F32)
            nc.vector.tensor_tensor(out=diff[:], in0=ap_d[:], in1=an_f[:],
                                    op=ALU.subtract)
            lvec = const.tile([128, MQ], F32)
            nc.scalar.activation(out=lvec[:], in_=diff[:], func=ACTF.Relu,
                                 bias=marg[:])
            lcol = const.tile([128, 1], F32)
            nc.vector.tensor_reduce(out=lcol[:], in_=lvec[:], axis=AX.X, op=ALU.add)
            lsum = const.tile([128, 1], F32)
            nc.gpsimd.partition_all_reduce(lsum[:], lcol[:], 128,
                                           bass_isa.ReduceOp.add)
            nc.sync.dma_start(out=loss_h[:], in_=lsum[0:1, 0:1])
            if dbg_h is not None:
                dbgt = const.tile([128, 8 * MQ], F32)
                nc.vector.tensor_copy(dbgt[:, 0:MQ], apw0[:])
                nc.vector.tensor_copy(dbgt[:, MQ:2 * MQ], apw1[:])
                nc.vector.tensor_copy(dbgt[:, 2 * MQ:3 * MQ], anmin[:])
                nc.vector.tensor_copy(dbgt[:, 3 * MQ:4 * MQ], wmin[:])
                nc.vector.tensor_copy(dbgt[:, 4 * MQ:5 * MQ], ap_d[:])
                nc.vector.tensor_copy(dbgt[:, 5 * MQ:6 * MQ], an_f[:])
                nc.vector.tensor_copy(dbgt[:, 6 * MQ:7 * MQ], lvec[:])
                nc.vector.tensor_copy(dbgt[:, 7 * MQ:8 * MQ], wb[:, 0:MQ])
                nc.sync.dma_start(out=dbg_h[:, :], in_=dbgt[:])

    nc.finalize()
    return nc


def _get_nc():
    global _nc_cache
    if _nc_cache is None:
        _nc_cache = _build()
    return _nc_cache


def _to_xT(xneg_rolled):
    # [N, D] key values -> [128, q*(KD*QW) + s*QW + j] quarter-blocked -X^T
    return np.ascontiguousarray(
        xneg_rolled.T.reshape(KD, 128, NQR, QW).transpose(1, 2, 0, 3)
        .reshape(128, KD * N)
    )


def _in_maps(inputs, targets, center):
    x = np.asarray(inputs, dtype=np.float32)
    t = np.asarray(targets).astype(np.int64).reshape(-1)
    c = np.ascontiguousarray(np.asarray(center, dtype=np.float32))
    assert x.shape == (N, D) and t.shape == (N,) and c.shape == (P, D)

    # sort rows by class (the loss is a mean over rows -> permutation
    # invariant); each query's same-class keys become one contiguous range
    perm = np.argsort(t, kind="stable")
    xs = x[perm]
    ts_ = t[perm]
    starts = np.searchsorted(ts_, ts_, side="left")
    ends = np.searchsorted(ts_, ts_, side="right")
    assert int((ends - starts).max()) <= 512, "class too large for max window"

    xneg = (-xs).astype(NPF8)                          # key values, fp8
    x2 = (2.0 * xs).astype(NPF8)                       # query values, fp8

    # aug row map: row 0 = sq_i, row 96 = sq_j/csq,
    # classes 0..94 -> rows 1..95, classes 95..99 -> rows 97..101
    rows = np.where(ts_ < 95, ts_ + 1, ts_ + 2)
    augk = np.zeros((128, N), dtype=NPBF16)
    augk[rows, np.arange(N)] = NPBF16(BIG)
    augk[0, :] = NPBF16(1.0)                           # sq_i coefficient

    # center aug rows: csq (=1) at row 96, huge for pad centers, sq_i coeff
    augc = np.zeros((128, 128), dtype=NPBF16)
    augc[96, 0:P] = NPBF16(1.0)
    augc[96, P:128] = NPBF16(1.0e6)
    augc[0, :] = NPBF16(1.0)

    maps = []
    for core in range(N_CORES):
        s = slice(core * NQ, (core + 1) * NQ)
        # rotate keys so this core's class neighborhood sits at cols [0,1536)
        shift = (core * NQ - 512) % N
        xT = _to_xT(np.roll(xneg, -shift, axis=0))
        augk_c = np.ascontiguousarray(np.roll(augk, -shift, axis=1))
        xq2T = np.ascontiguousarray(
            x2[s].T.reshape(KD, 128, NQ).transpose(1, 0, 2).reshape(128, KD * NQ)
        )
        augq = np.zeros((128, NQ), dtype=NPBF16)
        augq[rows[s], np.arange(NQ)] = NPBF16(1.0)
        augq[96, :] = NPBF16(1.0)                      # sq_j coefficient

        # window bounds per query row, in rotated coords, per chunk
        rl = (starts[s] - shift) % N
        rh = rl + (ends[s] - starts[s])
        assert rl.min() >= 0 and rh.max() <= 1536, "window escape"
        wb = np.zeros((128, MQ * 4), dtype=np.float32)
        for m in range(MQ):
            seg = slice(m * 128, (m + 1) * 128)
            wb[:, 4 * m + 0] = np.clip(rl[seg], 0, 1024)
            wb[:, 4 * m + 1] = np.clip(rh[seg], 0, 1024)
            wb[:, 4 * m + 2] = np.clip(rl[seg] - 1024, 0, 512)
            wb[:, 4 * m + 3] = np.clip(rh[seg] - 1024, 0, 512)

        maps.append({
            "xT": xT,
            "xq2T": xq2T,
            "augk": augk_c,
            "augq": augq,
            "augc": augc,
            "center": c,
            "wb": wb,
        })
    return maps


def run(inputs, targets, center, trace=False):
    nc = _get_nc()
    res = run_bass_kernel_spmd(
        nc, _in_maps(inputs, targets, center), list(range(N_CORES)), trace=trace
    )
    tot = sum(float(r["loss"][0, 0]) for r in res.results)
    loss = np.float32(tot / N)
    return np.asarray(loss), res


def kernel(inputs, targets, center):
    out, _ = run(inputs, targets, center)
    return out
